# revision 3
# baseline (speedup 1.0000x reference)
"""Trainium2 Bass kernel for nn_GCNN_87668872446200 (v3: branch-split + AllGather).

Two GCNConv+pool protein branches + two masif conv branches + dense head,
distributed over 8 NeuronCores as 2 branch-groups x 4 dest-node quarters.

Cores 0-3 handle protein branch 1 (quarters 0-3), cores 4-7 branch 2.
Per core (full 1024-dim features on the heavy paths, fp8):
  - xw = x_quarter @ W via fp8 DoubleRow matmuls for ONLY this core's 2560
    nodes -> local DRAM [2560, 1024] fp8 (no redundant compute)
  - ONE AllGather per 4-core branch group -> xw_full [10240, 1024] fp8
  - dma_gather pulls full 1KB source rows for this core's edge quarter
    (half the descriptors vs 512B rows, better HBM efficiency)
  - scatter-add as fp8 DoubleRow PE matmuls: S[256 edges, 128 dests]
    (host-built, norm-scaled, degree-balanced dest blocks) x gathered
    [256, 1024] accumulated in PSUM (2 x 512 halves)
  - h = lrelu(psum + bias) [128, 1024] bf16; transposed mean-pool via PE
    (pooledT [1024, 32]) folding 1/cnt
  - x_pre = W_pf^T @ pooledT partial [128, 32] (pre-activation, linear ->
    summable across the 4 quarter cores on host)
  - masif branch: 4 graphs/core (all cores, same as before)
  - out [128, 64]: cols 0:32 xpre partial, cols 32:64 masif partials
Host: sum partials per branch group, run the tiny dense head.

All 8 cores run ONE identical program; per-core variation is in input data.
"""
import numpy as np

# ---------------------------------------------------------------- constants
N_CORES = 8
N_QUART = 4       # dest-node quarters per branch group
P = 128
BLK = 128         # dest nodes per scatter block (S width)
PAIR_E = 256      # edges per DoubleRow matmul (2 chunks of 128)
GRP = 8           # chunks per dma_gather call (1024 idxs per call)
PAIRS_PER_CALL = GRP // 2

# problem sizes (hardcoded per spec)
N_NODES, N_EDGES, F_DIM, B_GRAPHS, L_MAS, C_MAS = 10000, 80000, 1024, 32, 800, 16


def _fp8():
    import ml_dtypes
    return ml_dtypes.float8_e4m3fn


def _bf16():
    import ml_dtypes
    return ml_dtypes.bfloat16


class _Cfg:
    def __init__(self, n=N_NODES, e=N_EDGES, f=F_DIM, b=B_GRAPHS,
                 l=L_MAS, c=C_MAS):
        assert f % 512 == 0 and b == 32 and l % 80 == 0 and c % 2 == 0
        self.N, self.E, self.F, self.B, self.L, self.C = n, e, f, b, l, c
        self.NPAD = ((n + 2047) // 2048) * 2048
        self.QUART = self.NPAD // N_QUART      # nodes per quarter (2560)
        assert self.QUART % 512 == 0
        self.NT = self.QUART // 512            # local node tiles (5)
        self.NBLK = self.QUART // BLK          # dest blocks per quarter (20)
        self.KC = f // P                       # k-chunks of contraction (8)
        self.KP = self.KC // 2                 # k-pairs (DoubleRow) (4)
        self.GPB = b // N_CORES                # graphs per core for masif
        self.LW = l // 80                      # avg-pool window (10)
        self.LB = 8                            # l-blocks for masif layout
        self.LBS = l // self.LB                # l-block size (100)
        assert self.LBS % self.LW == 0
        self.WPB = self.LBS // self.LW         # windows per l-block (10)


# ------------------------------------------------------------- arena layout
# (name, rows, dtype-key, shape) -- shared by host packer and kernel views
def _arena_layout(cfg):
    return [
        ('wg', 128, 'fp8', (cfg.KC, cfg.F)),
        ('bg', 128, 'f32', (cfg.F,)),
        ('mp', 128, 'bf16', (cfg.NBLK, cfg.B)),
        ('wpf', 128, 'f32', (cfg.KC, 128)),
        ('wm1', 10, 'f32', (8, 64)),
        ('wm2', 10, 'f32', (8, 64)),
        ('bm1', 64, 'f32', (1,)),
        ('bm2', 64, 'f32', (1,)),
        ('scale_s1', 32, 'f32', (1,)),
        ('bias_s1', 32, 'f32', (1,)),
        ('scale_f1', 32, 'f32', (1,)),
        ('bias_f1', 32, 'f32', (1,)),
        ('scale_s2', 32, 'f32', (1,)),
        ('bias_s2', 32, 'f32', (1,)),
        ('scale_f2', 32, 'f32', (1,)),
        ('bias_f2', 32, 'f32', (1,)),
        ('gmask', 64, 'f32', (cfg.B,)),
    ]


def _dt_size(key):
    return {'f32': 4, 'bf16': 2, 'fp8': 1}[key]


def _arena_offsets(cfg):
    off, out = 0, {}
    for name, rows, key, shape in _arena_layout(cfg):
        nb = int(np.prod(shape)) * _dt_size(key)
        out[name] = (off, rows, key, shape, nb)
        off += (nb + 63) // 64 * 64
    return out, off


# ---------------------------------------------------------------- host prep
def _edge_plan(cfg, edge_indices):
    """Shared block-pair profile (kp) + per-shard scatter plans.

    Returns (kp, sched, npairs, shards) where shards[(br, qu)] =
    (srcs [nchunk,128] int16, smat [nchunk,128,128] fp8, dest_of) and
    dest_of[(br, qu)] maps block,offset -> global dest node (for mp).
    """
    fp8 = _fp8()
    per_shard = {}
    for br in (1, 2):
        ei = edge_indices[br]
        row = np.asarray(ei[0]).astype(np.int64)
        col = np.asarray(ei[1]).astype(np.int64)
        loops = np.arange(cfg.N, dtype=np.int64)
        rows = np.concatenate([row, loops])
        cols = np.concatenate([col, loops])
        deg = np.bincount(cols, minlength=cfg.N).astype(np.float64)
        dinv = 1.0 / np.sqrt(deg)
        norm = (dinv[rows] * dinv[cols]).astype(np.float32)
        for qu in range(N_QUART):
            lo = qu * cfg.QUART
            sel = (cols >= lo) & (cols < lo + cfg.QUART)
            r, c, w = rows[sel], cols[sel] - lo, norm[sel]
            per_shard[(br, qu)] = (r, c, w)

    e_max = max(len(r) for r, _, _ in per_shard.values())
    npairs = -(-int(e_max * 1.02) // PAIR_E)
    npairs = -(-npairs // PAIRS_PER_CALL) * PAIRS_PER_CALL

    # find a feasible shared kp profile (block capacities) for all shards
    while True:
        base, extra = npairs // cfg.NBLK, npairs % cfg.NBLK
        kp = np.full(cfg.NBLK, base, np.int64)
        kp[:extra] += 1
        caps = kp * PAIR_E
        ok = True
        assigns = {}
        for key, (r, c, w) in per_shard.items():
            cnt = np.bincount(c, minlength=cfg.QUART)
            a = _pack_blocks(cfg, cnt, caps)
            if a is None:
                ok = False
                break
            assigns[key] = a
        if ok:
            break
        npairs += PAIRS_PER_CALL

    sched = []
    for j in range(cfg.NBLK):
        for t in range(kp[j]):
            sched.append((j, t == 0, t == int(kp[j]) - 1))
    assert len(sched) == npairs
    base_ch = np.zeros(cfg.NBLK, np.int64)
    base_ch[1:] = np.cumsum(2 * kp)[:-1]
    nchunk = 2 * npairs

    shards = {}
    for key, (r, c, w) in per_shard.items():
        blk_of, off_of = assigns[key]          # per local dest
        order = np.lexsort((off_of[c], blk_of[c]))
        r, c, w = r[order], c[order], w[order]
        b = blk_of[c]
        starts = np.searchsorted(b, np.arange(cfg.NBLK), 'left')
        o = np.arange(len(r)) - starts[b]
        ch = base_ch[b] + o // P
        sl = o % P
        srcs = np.zeros((nchunk, P), np.int16)
        smat = np.zeros((nchunk, P, BLK), fp8)
        srcs[ch, sl] = r.astype(np.int16)
        smat[ch, sl, off_of[c]] = w.astype(fp8)
        shards[key] = (srcs, smat, (blk_of, off_of))
    return kp, sched, npairs, shards


def _pack_blocks(cfg, cnt, caps):
    """Assign QUART dests into NBLK blocks of exactly BLK dests so block
    edge-counts fit caps. Greedy: heaviest dest -> block with most slack."""
    nb = cfg.NBLK
    order = np.argsort(-cnt, kind='stable')
    load = np.zeros(nb, np.int64)
    nmem = np.zeros(nb, np.int64)
    blk_of = np.zeros(cfg.QUART, np.int64)
    off_of = np.zeros(cfg.QUART, np.int64)
    for d in order:
        slack = caps - load - cnt[d]
        slack[nmem >= BLK] = np.iinfo(np.int64).min
        j = int(np.argmax(slack))
        if slack[j] < 0:
            return None
        blk_of[d] = j
        off_of[d] = nmem[j]
        load[j] += cnt[d]
        nmem[j] += 1
    assert (nmem == BLK).all()
    return blk_of, off_of


def _wrap_idxs(srcs):
    """[C, 128] int16 -> wrapped [128, C*8] (idx j at [j%16 + 16*rep, j//16])."""
    flat = srcs.reshape(-1)
    w = flat.reshape(-1, 16).T                # [16, C*8]
    return np.ascontiguousarray(np.tile(w, (8, 1)).astype(np.int16))


def _build_scs(srcs, smat):
    """Combine wrapped idxs + grouped smat into one [calls, 128, 1152] u8."""
    nchunk = srcs.shape[0]
    calls = nchunk // GRP
    idxw = _wrap_idxs(srcs)                   # [128, nchunk*8] int16
    scs = np.zeros((calls, P, 128 + GRP * BLK), np.uint8)
    idxu = idxw.view(np.uint8).reshape(P, calls, 128).transpose(1, 0, 2)
    scs[:, :, 0:128] = idxu
    smu = smat.view(np.uint8).reshape(calls, GRP, P, BLK)
    scs[:, :, 128:] = smu.transpose(0, 2, 1, 3).reshape(calls, P, GRP * BLK)
    return scs


def _mpool(cfg, batch, qu, assign):
    """[128, NBLK, B] bf16 folding 1/cnt, zero rows for pad nodes."""
    batch = np.asarray(batch).astype(np.int64)
    cnt = np.bincount(batch, minlength=cfg.B).astype(np.float64)
    cinv = (1.0 / np.maximum(cnt, 1.0)).astype(np.float32)
    blk_of, off_of = assign
    m = np.zeros((P, cfg.NBLK, cfg.B), np.float32)
    lo = qu * cfg.QUART
    hi = min(lo + cfg.QUART, cfg.N)
    if hi > lo:
        nodes = np.arange(lo, hi)
        rel = nodes - lo
        m[off_of[rel], blk_of[rel], batch[nodes]] = cinv[batch[nodes]]
    return m.astype(_bf16())


def _xtile(cfg, x, qu):
    """x quarter [QUART, F] f32 -> [NT, 128, KC, 512] fp8."""
    fp8 = _fp8()
    x8 = np.zeros((cfg.QUART, cfg.F), fp8)
    lo = qu * cfg.QUART
    hi = min(lo + cfg.QUART, cfg.N)
    x8[:hi - lo] = np.asarray(x[lo:hi], np.float32).astype(fp8)
    t = x8.reshape(cfg.NT, 512, cfg.KC, P).transpose(0, 3, 2, 1)
    return np.ascontiguousarray(t)


def _pack_arena(cfg, arrays):
    offs, total = _arena_offsets(cfg)
    ab = (total + 63) // 64 * 64
    arena = np.zeros((P, ab), np.uint8)
    for name, (off, rows, key, shape, nb) in offs.items():
        a = arrays[name]
        assert a.shape == (rows,) + tuple(shape), (name, a.shape, rows, shape)
        npdt = {'f32': np.float32, 'bf16': _bf16(), 'fp8': _fp8()}[key]
        flat = np.ascontiguousarray(a.astype(npdt)).view(np.uint8).reshape(rows, nb)
        arena[:rows, off:off + nb] = flat
    return arena


def _preprocess(inputs, cfg):
    fp8 = _fp8()
    kp, sched, npairs, shards = _edge_plan(
        cfg, {1: inputs['pro1_edge_index'], 2: inputs['pro2_edge_index']})
    meta = {'kp': tuple(int(v) for v in kp), 'sched': sched, 'npairs': npairs}

    def f32(v):
        return np.asarray(v, np.float32)

    in_maps = []
    for core in range(N_CORES):
        br, qu = core // N_QUART + 1, core % N_QUART
        ar = {}
        Wg = f32(inputs[f'W_g{br}'])
        ar['wg'] = np.ascontiguousarray(
            Wg.reshape(cfg.KC, P, cfg.F).transpose(1, 0, 2)).astype(fp8)
        ar['bg'] = np.tile(f32(inputs[f'b_g{br}'])[None, :], (P, 1))
        srcs, smat, assign = shards[(br, qu)]
        ar['mp'] = _mpool(cfg, inputs[f'pro{br}_batch'], qu, assign)
        Wpf = f32(inputs[f'W_pf{br}'])
        ar['wpf'] = np.ascontiguousarray(
            Wpf.reshape(cfg.KC, P, P).transpose(1, 0, 2))
        for mi in (1, 2):
            ar[f'wm{mi}'] = np.ascontiguousarray(
                (f32(inputs[f'W_m{mi}']) / (2.0 * cfg.LW))
                .reshape(8, 10, 64).transpose(1, 0, 2))
            ar[f'bm{mi}'] = f32(inputs[f'b_m{mi}']).reshape(64, 1)
            for sf, pre in (('s', 'cs'), ('f', 'cf')):
                w = float(np.asarray(inputs[f'{pre}{mi}_w'])[0])
                b = float(np.asarray(inputs[f'{pre}{mi}_b'])[0])
                ar[f'scale_{sf}{mi}'] = np.full((32, 1), w / cfg.C, np.float32)
                ar[f'bias_{sf}{mi}'] = np.full((32, 1), b, np.float32)
        gm = np.zeros((64, cfg.B), np.float32)
        gm[:, core * cfg.GPB:(core + 1) * cfg.GPB] = 1.0
        ar['gmask'] = gm

        arena = _pack_arena(cfg, ar)

        # masif arena2: 4 tensors x [32, 16, 100] f32 = 4 x 6400B per row
        a2 = np.zeros((32, 4 * 6400), np.uint8)
        for ti, name in enumerate(['mas1_straight', 'mas1_flipped',
                                   'mas2_straight', 'mas2_flipped']):
            a = f32(inputs[name])[core * cfg.GPB:(core + 1) * cfg.GPB]
            blk = a.reshape(cfg.GPB, cfg.C, cfg.LB, cfg.LBS) \
                   .transpose(2, 0, 1, 3).reshape(32, cfg.C * cfg.LBS)
            a2[:, ti * 6400:(ti + 1) * 6400] = \
                np.ascontiguousarray(blk).view(np.uint8)

        m = {'arena': arena, 'arena2': a2,
             'xT': _xtile(cfg, inputs[f'pro{br}_x'], qu),
             'scs': _build_scs(srcs, smat)}
        in_maps.append(m)
    return meta, in_maps


# ---------------------------------------------------------------- program
def _build(cfg, meta):
    import concourse.bass as bass
    import concourse.bacc as bacc
    import concourse.mybir as mybir
    import concourse.tile as tile
    from concourse.masks import make_identity

    dt = mybir.dt
    fp8 = dt.float8e4
    bf16 = dt.bfloat16
    f32 = dt.float32
    u8 = dt.uint8
    AF = mybir.ActivationFunctionType
    OP = mybir.AluOpType
    DR = mybir.MatmulPerfMode.DoubleRow

    nc = bacc.Bacc("TRN2", target_bir_lowering=False, debug=False,
                   enable_asserts=False, num_devices=N_CORES,
                   num_swdge_queues=2)

    offs, total = _arena_offsets(cfg)
    AB = (total + 63) // 64 * 64
    npairs = meta['npairs']
    sched = meta['sched']
    n_call = npairs // PAIRS_PER_CALL

    arena_d = nc.dram_tensor('arena', [P, AB], u8, kind="ExternalInput")
    arena2_d = nc.dram_tensor('arena2', [32, 4 * 6400], u8, kind="ExternalInput")
    xT_d = nc.dram_tensor('xT', [cfg.NT, P, cfg.KC, 512], fp8,
                          kind="ExternalInput")
    scs_d = nc.dram_tensor('scs', [n_call, P, 128 + GRP * BLK], u8,
                           kind="ExternalInput")
    out_t = nc.dram_tensor('out', [P, 64], f32, kind="ExternalOutput")

    with tile.TileContext(nc) as tc:
        with tc.tile_pool(name="const", bufs=1) as cst, \
             tc.tile_pool(name="xt", bufs=3) as xtp, \
             tc.tile_pool(name="xwps", bufs=2, space="PSUM") as xwps, \
             tc.tile_pool(name="xwsb", bufs=2) as xwsb, \
             tc.tile_pool(name="scs", bufs=12) as scsp, \
             tc.tile_pool(name="gat", bufs=6) as gatp, \
             tc.tile_pool(name="blkpsA", bufs=2, space="PSUM") as blkpsA, \
             tc.tile_pool(name="blkpsB", bufs=2, space="PSUM") as blkpsB, \
             tc.tile_pool(name="hb", bufs=3) as hp, \
             tc.tile_pool(name="poolps", bufs=1, space="PSUM") as poolp, \
             tc.tile_pool(name="small", bufs=2) as smp, \
             tc.tile_pool(name="smallps", bufs=1, space="PSUM") as smps, \
             tc.tile_pool(name="dram", bufs=1, space="DRAM") as drp:

            # ---------------- constant arena (one DMA each)
            arena_t = cst.tile([P, AB], u8, tag='arena')
            nc.sync.dma_start(out=arena_t[:], in_=arena_d.ap())
            arena2_t = cst.tile([32, 4 * 6400], u8, tag='arena2')
            nc.sync.dma_start(out=arena2_t[:], in_=arena2_d.ap())

            def av(name, dtype):
                off, rows, key, shape, nb = offs[name]
                v = arena_t[0:rows, off:off + nb].bitcast(dtype)
                if len(shape) == 2:
                    v = v.rearrange("p (a b) -> p a b", a=shape[0])
                return v

            wg_v = av('wg', fp8)
            bg_v = av('bg', f32)
            mp_v = av('mp', bf16)
            wpf_v = av('wpf', f32)
            wm_v = {mi: av(f'wm{mi}', f32) for mi in (1, 2)}
            bm_v = {mi: av(f'bm{mi}', f32) for mi in (1, 2)}
            msc_v = {(mi, sf, kind): av(f'{kind}_{sf}{mi}', f32)
                     for mi in (1, 2) for sf in 'sf'
                     for kind in ('scale', 'bias')}
            gmask_v = av('gmask', f32)

            id32 = cst.tile([32, 32], f32, tag='id32')
            make_identity(nc, id32[:])

            # ---------------- masif (both branches) -> two [64, B] f32 tiles
            for mi in (1, 2):
                frag = None
                for si, sf in enumerate('sf'):
                    toff = ((mi - 1) * 2 + si) * 6400
                    mv = arena2_t[:, toff:toff + 6400].bitcast(f32) \
                        .rearrange("p (c l) -> p c l", c=cfg.C)
                    red = smp.tile([32, cfg.LBS], f32, tag='masred')
                    nc.vector.tensor_reduce(
                        out=red[:], in_=mv.transpose([0, 2, 1]),
                        axis=mybir.AxisListType.X, op=OP.add)
                    act = smp.tile([32, cfg.LBS], f32, tag='masact')
                    nc.scalar.activation(
                        act[:], red[:], AF.Relu,
                        bias=msc_v[(mi, sf, 'bias')][:, 0:1],
                        scale=msc_v[(mi, sf, 'scale')][:, 0:1])
                    ws = smp.tile([32, cfg.WPB], f32, tag='masws')
                    nc.vector.tensor_reduce(
                        out=ws[:],
                        in_=act[:].rearrange("p (w l) -> p w l", l=cfg.LW),
                        axis=mybir.AxisListType.X, op=OP.add)
                    if frag is None:
                        frag = ws
                    else:
                        frag2 = smp.tile([32, cfg.WPB], f32, tag='masfrag')
                        nc.vector.tensor_add(out=frag2[:], in0=frag[:],
                                             in1=ws[:])
                        frag = frag2
                ps_t = smps.tile([cfg.WPB, 32], f32, space="PSUM", tag='sps')
                nc.tensor.transpose(out=ps_t[:], in_=frag[:], identity=id32[:])
                fragT = smp.tile([cfg.WPB, 32], f32, tag='masfragT')
                nc.scalar.activation(fragT[:], ps_t[:], AF.Identity)
                fragTc = fragT[:].rearrange("k (lb g) -> k lb g", g=cfg.GPB)
                m_ps = smps.tile([64, cfg.GPB], f32, space="PSUM", tag='sps')
                for lb in range(cfg.LB):
                    nc.tensor.matmul(
                        m_ps[:], lhsT=wm_v[mi][:, lb, :], rhs=fragTc[:, lb, :],
                        start=(lb == 0), stop=(lb == cfg.LB - 1))
                m_fm = smp.tile([64, cfg.GPB], f32, tag='masfm')
                nc.scalar.activation(m_fm[:], m_ps[:], AF.Identity,
                                     bias=bm_v[mi][:, 0:1])
                masm = cst.tile([64, cfg.B], f32, tag=f'masasm{mi}')
                nc.vector.tensor_tensor(
                    out=masm[:].rearrange("p (s g) -> p s g", g=cfg.GPB),
                    in0=m_fm[:, None, :].to_broadcast(
                        [64, N_CORES, cfg.GPB]),
                    in1=gmask_v.rearrange("p (s g) -> p s g", g=cfg.GPB),
                    op=OP.mult)
                nc.sync.dma_start(
                    out=out_t.ap()[(mi - 1) * 64:mi * 64, 32:64], in_=masm[:])

            # ---------------- GCN branch (this core's branch + quarter)
            xw_loc = drp.tile([cfg.QUART, cfg.F], fp8, tag='xwloc')
            xw_full = drp.tile([cfg.NPAD, cfg.F], fp8, tag='xwfull')

            # xw = x_quarter @ W (fp8 DoubleRow, x stationary)
            for nt in range(cfg.NT):
                xt_t = xtp.tile([P, cfg.KC, 512], fp8, tag='xt')
                nc.sync.dma_start(out=xt_t[:], in_=xT_d.ap()[nt])
                xw_t = xwsb.tile([P, 4, cfg.F], fp8, tag='xwsb')
                for sub in range(4):
                    for hf in range(2):
                        ps = xwps.tile([P, 512], f32, space="PSUM",
                                       tag='xwps')
                        for kpi in range(cfg.KP):
                            nc.tensor.matmul(
                                ps[:],
                                lhsT=xt_t[:, 2 * kpi:2 * kpi + 2,
                                          sub * P:(sub + 1) * P],
                                rhs=wg_v[:, 2 * kpi:2 * kpi + 2,
                                         hf * 512:(hf + 1) * 512],
                                start=(kpi == 0), stop=(kpi == cfg.KP - 1),
                                perf_mode=DR)
                        if (sub + hf) % 2 == 0:
                            nc.vector.tensor_scalar_mul(
                                xw_t[:, sub, hf * 512:(hf + 1) * 512],
                                ps[:], 1.0)
                        else:
                            nc.scalar.activation(
                                xw_t[:, sub, hf * 512:(hf + 1) * 512],
                                ps[:], AF.Identity)
                nc.sync.dma_start(
                    out=xw_loc[nt * 512:(nt + 1) * 512, :].rearrange(
                        "(s p) f -> p s f", p=P),
                    in_=xw_t[:])

            # AllGather xw across this core's branch group
            nc.gpsimd.collective_compute(
                "AllGather", mybir.AluOpType.bypass,
                replica_groups=[[0, 1, 2, 3], [4, 5, 6, 7]],
                ins=[xw_loc[:]],
                outs=[xw_full[:]])

            # scatter + transposed pool
            poolps = poolp.tile([P, cfg.KC, cfg.B], f32, space="PSUM",
                                tag='poolps')
            pi = 0
            psA = psB = None
            for g in range(n_call):
                scs_t = scsp.tile([P, 128 + GRP * BLK], u8, tag='scs')
                nc.sync.dma_start(out=scs_t[:], in_=scs_d.ap()[g])
                idx_v = scs_t[:, 0:128].bitcast(dt.int16)
                s_v = scs_t[:, 128:128 + GRP * BLK].bitcast(fp8) \
                    .rearrange("p (c d) -> p c d", c=GRP)
                gat_t = gatp.tile([P, GRP, cfg.F], fp8, tag='gat')
                nc.gpsimd.dma_gather(
                    out_ap=gat_t[:], in_ap=xw_full[:],
                    idxs_ap=idx_v,
                    num_idxs=GRP * P, num_idxs_reg=GRP * P,
                    elem_size=cfg.F, queue_num=g % 2)
                for i in range(PAIRS_PER_CALL):
                    j, st, sp = sched[pi]
                    if st:
                        psA = blkpsA.tile([P, 512], f32, space="PSUM",
                                          tag='blkA')
                        psB = blkpsB.tile([P, 512], f32, space="PSUM",
                                          tag='blkB')
                    nc.tensor.matmul(
                        psA[:],
                        lhsT=s_v[:, 2 * i:2 * i + 2, :],
                        rhs=gat_t[:, 2 * i:2 * i + 2, 0:512],
                        start=st, stop=sp, perf_mode=DR)
                    nc.tensor.matmul(
                        psB[:],
                        lhsT=s_v[:, 2 * i:2 * i + 2, :],
                        rhs=gat_t[:, 2 * i:2 * i + 2, 512:1024],
                        start=st, stop=sp, perf_mode=DR)
                    if sp:
                        h_t = hp.tile([P, cfg.F], bf16, tag='h')
                        nc.vector.tensor_add(out=h_t[:, 0:512], in0=psA[:],
                                             in1=bg_v[:, 0:512])
                        nc.vector.tensor_add(out=h_t[:, 512:1024], in0=psB[:],
                                             in1=bg_v[:, 512:1024])
                        nc.scalar.activation(h_t[:], h_t[:], AF.Lrelu,
                                             alpha=0.01)
                        for c in range(cfg.KC):
                            nc.tensor.matmul(
                                poolps[:, c, :],
                                lhsT=h_t[:, c * P:(c + 1) * P],
                                rhs=mp_v[:, j, :],
                                start=(j == 0), stop=(j == cfg.NBLK - 1),
                                skip_group_check=True)
                    pi += 1

            # x_pre partial: [128, B] = W_pf^T @ pooledT
            pooled_sb = smp.tile([P, cfg.KC, cfg.B], f32, tag='pooled')
            nc.vector.tensor_scalar_mul(pooled_sb[:], poolps[:], 1.0)
            xpre_ps = smps.tile([P, cfg.B], f32, space="PSUM", tag='sps')
            for c in range(cfg.KC):
                nc.tensor.matmul(xpre_ps[:], lhsT=wpf_v[:, c, :],
                                 rhs=pooled_sb[:, c, :],
                                 start=(c == 0), stop=(c == cfg.KC - 1))
            xpre_sb = smp.tile([P, cfg.B], f32, tag='xpresb')
            nc.vector.tensor_scalar_mul(xpre_sb[:], xpre_ps[:], 1.0)
            nc.sync.dma_start(out=out_t.ap()[:, 0:32], in_=xpre_sb[:])

    nc.compile()
    return nc


# ---------------------------------------------------------------- entry
_CACHE = {}


def _run(inputs, cfg, trace=False, tmpdir=None):
    from concourse import bass_utils
    meta, in_maps = _preprocess(inputs, cfg)
    key = (cfg.N, cfg.F, meta['npairs'], meta['kp'])
    if key not in _CACHE:
        _CACHE.clear()
        _CACHE[key] = _build(cfg, meta)
    nc = _CACHE[key]
    res = bass_utils.run_bass_kernel_spmd(
        nc, in_maps, core_ids=list(range(N_CORES)), trace=trace, tmpdir=tmpdir)
    outs = [np.asarray(res.results[i]['out'], np.float64)
            for i in range(N_CORES)]
    out = _host_head(inputs, cfg, outs)
    return out, res


def _lrelu(v):
    return np.where(v > 0, v, 0.01 * v)


def _host_head(inputs, cfg, outs):
    """Unshard: sum of per-core partials -> tiny dense head (host)."""
    f32 = np.float32
    xr1 = sum(outs[i][:, 0:32] for i in range(4))
    xr2 = sum(outs[i][:, 0:32] for i in range(4, 8))
    m1 = sum(outs[i][0:64, 32:64] for i in range(N_CORES))
    m2 = sum(outs[i][64:128, 32:64] for i in range(N_CORES))
    x1 = _lrelu(xr1 + np.asarray(inputs['b_pf1'], f32)[:, None])
    x2 = _lrelu(xr2 + np.asarray(inputs['b_pf2'], f32)[:, None])
    xcat = np.concatenate([x1, x2], 0)                       # [256, B]
    xc1 = _lrelu(np.asarray(inputs['W_fc1'], f32).T @ xcat
                 + np.asarray(inputs['b_fc1'], f32)[:, None])
    xc2 = _lrelu(np.asarray(inputs['W_fc2'], f32).T @ xc1
                 + np.asarray(inputs['b_fc2'], f32)[:, None])
    W_out = np.asarray(inputs['W_out'], f32)
    z = (W_out[0:64].T @ xc2 + W_out[64:128].T @ m1 + W_out[128:192].T @ m2
         + np.asarray(inputs['b_out'], f32)[:, None])
    return (1.0 / (1.0 + np.exp(-z))).T.astype(f32)         # [B, 1]


def kernel(**inputs) -> np.ndarray:
    cfg = _Cfg()
    out, _ = _run(inputs, cfg)
    return out


# revision 8
# speedup vs baseline: 1.1509x; 1.1509x over previous
"""Trainium2 Bass kernel for nn_GCNN_87668872446200 (v3: branch-split + AllGather).

Two GCNConv+pool protein branches + two masif conv branches + dense head,
distributed over 8 NeuronCores as 2 branch-groups x 4 dest-node quarters.

Cores 0-3 handle protein branch 1 (quarters 0-3), cores 4-7 branch 2.
Per core (full 1024-dim features on the heavy paths, fp8):
  - xw = x_quarter @ W via fp8 DoubleRow matmuls for ONLY this core's 2560
    nodes -> local DRAM [2560, 1024] fp8 (no redundant compute)
  - ONE AllGather per 4-core branch group -> xw_full [10240, 1024] fp8
  - dma_gather pulls full 1KB source rows for this core's edge quarter
    (half the descriptors vs 512B rows, better HBM efficiency)
  - scatter-add as fp8 DoubleRow PE matmuls: S[256 edges, 128 dests]
    (host-built, norm-scaled, degree-balanced dest blocks) x gathered
    [256, 1024] accumulated in PSUM (2 x 512 halves)
  - h = lrelu(psum + bias) [128, 1024] bf16; transposed mean-pool via PE
    (pooledT [1024, 32]) folding 1/cnt
  - x_pre = W_pf^T @ pooledT partial [128, 32] (pre-activation, linear ->
    summable across the 4 quarter cores on host)
  - masif branch: 4 graphs/core (all cores, same as before)
  - out [128, 64]: cols 0:32 xpre partial, cols 32:64 masif partials
Host: sum partials per branch group, run the tiny dense head.

All 8 cores run ONE identical program; per-core variation is in input data.
"""
import numpy as np

# ---------------------------------------------------------------- constants
N_CORES = 8
N_QUART = 4       # dest-node quarters per branch group
P = 128
BLK = 128         # dest nodes per scatter block (S width)
PAIR_E = 256      # edges per DoubleRow matmul (2 chunks of 128)
GRP = 8           # chunks per dma_gather call (1024 idxs per call)
PAIRS_PER_CALL = GRP // 2

# problem sizes (hardcoded per spec)
N_NODES, N_EDGES, F_DIM, B_GRAPHS, L_MAS, C_MAS = 10000, 80000, 1024, 32, 800, 16


def _fp8():
    import ml_dtypes
    return ml_dtypes.float8_e4m3fn


def _bf16():
    import ml_dtypes
    return ml_dtypes.bfloat16


class _Cfg:
    def __init__(self, n=N_NODES, e=N_EDGES, f=F_DIM, b=B_GRAPHS,
                 l=L_MAS, c=C_MAS):
        assert f % 512 == 0 and b == 32 and l % 80 == 0 and c % 2 == 0
        self.N, self.E, self.F, self.B, self.L, self.C = n, e, f, b, l, c
        self.NPAD = ((n + 2047) // 2048) * 2048
        self.QUART = self.NPAD // N_QUART      # nodes per quarter (2560)
        assert self.QUART % 512 == 0
        self.NT = self.QUART // 512            # local node tiles (5)
        self.NBLK = self.QUART // BLK          # dest blocks per quarter (20)
        self.KC = f // P                       # k-chunks of contraction (8)
        self.KP = self.KC // 2                 # k-pairs (DoubleRow) (4)
        self.GPB = b // N_CORES                # graphs per core for masif
        self.LW = l // 80                      # avg-pool window (10)
        self.LB = 8                            # l-blocks for masif layout
        self.LBS = l // self.LB                # l-block size (100)
        assert self.LBS % self.LW == 0
        self.WPB = self.LBS // self.LW         # windows per l-block (10)


# ------------------------------------------------------------- arena layout
# (name, rows, dtype-key, shape) -- shared by host packer and kernel views
def _arena_layout(cfg):
    return [
        ('wg', 128, 'fp8', (cfg.KC, cfg.F)),
        ('bg', 128, 'f32', (cfg.F,)),
        ('mp', 128, 'bf16', (cfg.NBLK, cfg.B)),
        ('wpf', 128, 'f32', (cfg.KC, 128)),
        ('wm1', 10, 'f32', (8, 64)),
        ('wm2', 10, 'f32', (8, 64)),
        ('bm1', 64, 'f32', (1,)),
        ('bm2', 64, 'f32', (1,)),
        ('scale_s1', 32, 'f32', (1,)),
        ('bias_s1', 32, 'f32', (1,)),
        ('scale_f1', 32, 'f32', (1,)),
        ('bias_f1', 32, 'f32', (1,)),
        ('scale_s2', 32, 'f32', (1,)),
        ('bias_s2', 32, 'f32', (1,)),
        ('scale_f2', 32, 'f32', (1,)),
        ('bias_f2', 32, 'f32', (1,)),
        ('gmask', 64, 'f32', (cfg.B,)),
    ]


def _dt_size(key):
    return {'f32': 4, 'bf16': 2, 'fp8': 1}[key]


def _arena_offsets(cfg):
    off, out = 0, {}
    for name, rows, key, shape in _arena_layout(cfg):
        nb = int(np.prod(shape)) * _dt_size(key)
        out[name] = (off, rows, key, shape, nb)
        off += (nb + 63) // 64 * 64
    return out, off


# ---------------------------------------------------------------- host prep
def _edge_plan(cfg, edge_indices):
    """Shared block-pair profile (kp) + per-shard scatter plans.

    Returns (kp, sched, npairs, shards) where shards[(br, qu)] =
    (srcs [nchunk,128] int16, smat [nchunk,128,128] fp8, dest_of) and
    dest_of[(br, qu)] maps block,offset -> global dest node (for mp).
    """
    fp8 = _fp8()
    per_shard = {}
    for br in (1, 2):
        ei = edge_indices[br]
        row = np.asarray(ei[0]).astype(np.int64)
        col = np.asarray(ei[1]).astype(np.int64)
        loops = np.arange(cfg.N, dtype=np.int64)
        rows = np.concatenate([row, loops])
        cols = np.concatenate([col, loops])
        deg = np.bincount(cols, minlength=cfg.N).astype(np.float64)
        dinv = 1.0 / np.sqrt(deg)
        norm = (dinv[rows] * dinv[cols]).astype(np.float32)
        for qu in range(N_QUART):
            lo = qu * cfg.QUART
            sel = (cols >= lo) & (cols < lo + cfg.QUART)
            # gather rows live in the 8-core shared buffer: branch 2 at +NPAD
            r = rows[sel] + (br - 1) * cfg.NPAD
            per_shard[(br, qu)] = (r, cols[sel] - lo, norm[sel])

    e_max = max(len(r) for r, _, _ in per_shard.values())
    npairs = -(-int(e_max * 1.02) // PAIR_E)
    npairs = -(-npairs // PAIRS_PER_CALL) * PAIRS_PER_CALL

    # find a feasible shared kp profile (block capacities) for all shards
    while True:
        base, extra = npairs // cfg.NBLK, npairs % cfg.NBLK
        kp = np.full(cfg.NBLK, base, np.int64)
        kp[:extra] += 1
        caps = kp * PAIR_E
        ok = True
        assigns = {}
        for key, (r, c, w) in per_shard.items():
            cnt = np.bincount(c, minlength=cfg.QUART)
            a = _pack_blocks(cfg, cnt, caps)
            if a is None:
                ok = False
                break
            assigns[key] = a
        if ok:
            break
        npairs += PAIRS_PER_CALL

    sched = []
    for j in range(cfg.NBLK):
        for t in range(kp[j]):
            sched.append((j, t == 0, t == int(kp[j]) - 1))
    assert len(sched) == npairs
    base_ch = np.zeros(cfg.NBLK, np.int64)
    base_ch[1:] = np.cumsum(2 * kp)[:-1]
    nchunk = 2 * npairs

    shards = {}
    for key, (r, c, w) in per_shard.items():
        blk_of, off_of = assigns[key]          # per local dest
        order = np.lexsort((off_of[c], blk_of[c]))
        r, c, w = r[order], c[order], w[order]
        b = blk_of[c]
        starts = np.searchsorted(b, np.arange(cfg.NBLK), 'left')
        o = np.arange(len(r)) - starts[b]
        ch = base_ch[b] + o // P
        sl = o % P
        srcs = np.zeros((nchunk, P), np.int16)
        smat = np.zeros((nchunk, P, BLK), fp8)
        srcs[ch, sl] = r.astype(np.int16)
        smat[ch, sl, off_of[c]] = w.astype(fp8)
        shards[key] = (srcs, smat, (blk_of, off_of))
    return kp, sched, npairs, shards


def _pack_blocks(cfg, cnt, caps):
    """Assign QUART dests into NBLK blocks of exactly BLK dests so block
    edge-counts fit caps. Greedy: heaviest dest -> block with most slack."""
    nb = cfg.NBLK
    order = np.argsort(-cnt, kind='stable')
    load = np.zeros(nb, np.int64)
    nmem = np.zeros(nb, np.int64)
    blk_of = np.zeros(cfg.QUART, np.int64)
    off_of = np.zeros(cfg.QUART, np.int64)
    for d in order:
        slack = caps - load - cnt[d]
        slack[nmem >= BLK] = np.iinfo(np.int64).min
        j = int(np.argmax(slack))
        if slack[j] < 0:
            return None
        blk_of[d] = j
        off_of[d] = nmem[j]
        load[j] += cnt[d]
        nmem[j] += 1
    assert (nmem == BLK).all()
    return blk_of, off_of


def _wrap_idxs(srcs):
    """[C, 128] int16 -> wrapped [128, C*8] (idx j at [j%16 + 16*rep, j//16])."""
    flat = srcs.reshape(-1)
    w = flat.reshape(-1, 16).T                # [16, C*8]
    return np.ascontiguousarray(np.tile(w, (8, 1)).astype(np.int16))


def _build_scs(srcs, smat):
    """Combine wrapped idxs + grouped smat into one [calls, 128, 1152] u8."""
    nchunk = srcs.shape[0]
    calls = nchunk // GRP
    idxw = _wrap_idxs(srcs)                   # [128, nchunk*8] int16
    scs = np.zeros((calls, P, 128 + GRP * BLK), np.uint8)
    idxu = idxw.view(np.uint8).reshape(P, calls, 128).transpose(1, 0, 2)
    scs[:, :, 0:128] = idxu
    smu = smat.view(np.uint8).reshape(calls, GRP, P, BLK)
    scs[:, :, 128:] = smu.transpose(0, 2, 1, 3).reshape(calls, P, GRP * BLK)
    return scs


def _mpool(cfg, batch, qu, assign):
    """[128, NBLK, B] bf16 folding 1/cnt, zero rows for pad nodes."""
    batch = np.asarray(batch).astype(np.int64)
    cnt = np.bincount(batch, minlength=cfg.B).astype(np.float64)
    cinv = (1.0 / np.maximum(cnt, 1.0)).astype(np.float32)
    blk_of, off_of = assign
    m = np.zeros((P, cfg.NBLK, cfg.B), np.float32)
    lo = qu * cfg.QUART
    hi = min(lo + cfg.QUART, cfg.N)
    if hi > lo:
        nodes = np.arange(lo, hi)
        rel = nodes - lo
        m[off_of[rel], blk_of[rel], batch[nodes]] = cinv[batch[nodes]]
    return m.astype(_bf16())


def _xtile(cfg, x, qu):
    """x quarter [QUART, F] f32 -> [NT, 128, KC, 512] fp8."""
    fp8 = _fp8()
    x8 = np.zeros((cfg.QUART, cfg.F), fp8)
    lo = qu * cfg.QUART
    hi = min(lo + cfg.QUART, cfg.N)
    x8[:hi - lo] = np.asarray(x[lo:hi], np.float32).astype(fp8)
    t = x8.reshape(cfg.NT, 512, cfg.KC, P).transpose(0, 3, 2, 1)
    return np.ascontiguousarray(t)


def _pack_arena(cfg, arrays):
    offs, total = _arena_offsets(cfg)
    ab = (total + 63) // 64 * 64
    arena = np.zeros((P, ab), np.uint8)
    for name, (off, rows, key, shape, nb) in offs.items():
        a = arrays[name]
        assert a.shape == (rows,) + tuple(shape), (name, a.shape, rows, shape)
        npdt = {'f32': np.float32, 'bf16': _bf16(), 'fp8': _fp8()}[key]
        flat = np.ascontiguousarray(a.astype(npdt)).view(np.uint8).reshape(rows, nb)
        arena[:rows, off:off + nb] = flat
    return arena


def _preprocess(inputs, cfg):
    fp8 = _fp8()
    kp, sched, npairs, shards = _edge_plan(
        cfg, {1: inputs['pro1_edge_index'], 2: inputs['pro2_edge_index']})
    meta = {'kp': tuple(int(v) for v in kp), 'sched': sched, 'npairs': npairs}

    def f32(v):
        return np.asarray(v, np.float32)

    in_maps = []
    for core in range(N_CORES):
        br, qu = core // N_QUART + 1, core % N_QUART
        ar = {}
        Wg = f32(inputs[f'W_g{br}'])
        ar['wg'] = np.ascontiguousarray(
            Wg.reshape(cfg.KC, P, cfg.F).transpose(1, 0, 2)).astype(fp8)
        ar['bg'] = np.tile(f32(inputs[f'b_g{br}'])[None, :], (P, 1))
        srcs, smat, assign = shards[(br, qu)]
        ar['mp'] = _mpool(cfg, inputs[f'pro{br}_batch'], qu, assign)
        Wpf = f32(inputs[f'W_pf{br}'])
        ar['wpf'] = np.ascontiguousarray(
            Wpf.reshape(cfg.KC, P, P).transpose(1, 0, 2))
        for mi in (1, 2):
            ar[f'wm{mi}'] = np.ascontiguousarray(
                (f32(inputs[f'W_m{mi}']) / (2.0 * cfg.LW))
                .reshape(8, 10, 64).transpose(1, 0, 2))
            ar[f'bm{mi}'] = f32(inputs[f'b_m{mi}']).reshape(64, 1)
            for sf, pre in (('s', 'cs'), ('f', 'cf')):
                w = float(np.asarray(inputs[f'{pre}{mi}_w'])[0])
                b = float(np.asarray(inputs[f'{pre}{mi}_b'])[0])
                ar[f'scale_{sf}{mi}'] = np.full((32, 1), w / cfg.C, np.float32)
                ar[f'bias_{sf}{mi}'] = np.full((32, 1), b, np.float32)
        gm = np.zeros((64, cfg.B), np.float32)
        gm[:, core * cfg.GPB:(core + 1) * cfg.GPB] = 1.0
        ar['gmask'] = gm

        arena = _pack_arena(cfg, ar)

        # masif arena2: 4 tensors x [32, 16, 100] f32 = 4 x 6400B per row
        a2 = np.zeros((32, 4 * 6400), np.uint8)
        for ti, name in enumerate(['mas1_straight', 'mas1_flipped',
                                   'mas2_straight', 'mas2_flipped']):
            a = f32(inputs[name])[core * cfg.GPB:(core + 1) * cfg.GPB]
            blk = a.reshape(cfg.GPB, cfg.C, cfg.LB, cfg.LBS) \
                   .transpose(2, 0, 1, 3).reshape(32, cfg.C * cfg.LBS)
            a2[:, ti * 6400:(ti + 1) * 6400] = \
                np.ascontiguousarray(blk).view(np.uint8)

        m = {'arena': arena, 'arena2': a2,
             'xT': _xtile(cfg, inputs[f'pro{br}_x'], qu),
             'scs': _build_scs(srcs, smat)}
        in_maps.append(m)
    return meta, in_maps


# ---------------------------------------------------------------- program
def _build(cfg, meta):
    import concourse.bass as bass
    import concourse.bacc as bacc
    import concourse.mybir as mybir
    import concourse.tile as tile
    from concourse.masks import make_identity

    dt = mybir.dt
    fp8 = dt.float8e4
    bf16 = dt.bfloat16
    f32 = dt.float32
    u8 = dt.uint8
    AF = mybir.ActivationFunctionType
    OP = mybir.AluOpType
    DR = mybir.MatmulPerfMode.DoubleRow

    nc = bacc.Bacc("TRN2", target_bir_lowering=False, debug=False,
                   enable_asserts=False, num_devices=N_CORES,
                   num_swdge_queues=2)

    offs, total = _arena_offsets(cfg)
    AB = (total + 63) // 64 * 64
    npairs = meta['npairs']
    sched = meta['sched']
    n_call = npairs // PAIRS_PER_CALL

    arena_d = nc.dram_tensor('arena', [P, AB], u8, kind="ExternalInput")
    arena2_d = nc.dram_tensor('arena2', [32, 4 * 6400], u8, kind="ExternalInput")
    xT_d = nc.dram_tensor('xT', [cfg.NT, P, cfg.KC, 512], fp8,
                          kind="ExternalInput")
    scs_d = nc.dram_tensor('scs', [n_call, P, 128 + GRP * BLK], u8,
                           kind="ExternalInput")
    out_t = nc.dram_tensor('out', [P, 64], f32, kind="ExternalOutput")
    xw_all = nc.dram_tensor('xw_all', [2 * cfg.NPAD, cfg.F], fp8,
                            kind="Internal", addr_space="Shared")

    with tile.TileContext(nc) as tc:
        with tc.tile_pool(name="const", bufs=1) as cst, \
             tc.tile_pool(name="xt", bufs=3) as xtp, \
             tc.tile_pool(name="xwps", bufs=2, space="PSUM") as xwps, \
             tc.tile_pool(name="xwsb", bufs=2) as xwsb, \
             tc.tile_pool(name="scs", bufs=12) as scsp, \
             tc.tile_pool(name="gat", bufs=6) as gatp, \
             tc.tile_pool(name="blkpsA", bufs=2, space="PSUM") as blkpsA, \
             tc.tile_pool(name="blkpsB", bufs=2, space="PSUM") as blkpsB, \
             tc.tile_pool(name="hb", bufs=3) as hp, \
             tc.tile_pool(name="poolps", bufs=1, space="PSUM") as poolp, \
             tc.tile_pool(name="small", bufs=2) as smp, \
             tc.tile_pool(name="smallps", bufs=1, space="PSUM") as smps, \
             tc.tile_pool(name="dram", bufs=1, space="DRAM") as drp:

            # ---------------- constant arena (one DMA each)
            arena_t = cst.tile([P, AB], u8, tag='arena')
            nc.sync.dma_start(out=arena_t[:], in_=arena_d.ap())
            arena2_t = cst.tile([32, 4 * 6400], u8, tag='arena2')
            nc.sync.dma_start(out=arena2_t[:], in_=arena2_d.ap())

            def av(name, dtype):
                off, rows, key, shape, nb = offs[name]
                v = arena_t[0:rows, off:off + nb].bitcast(dtype)
                if len(shape) == 2:
                    v = v.rearrange("p (a b) -> p a b", a=shape[0])
                return v

            wg_v = av('wg', fp8)
            bg_v = av('bg', f32)
            mp_v = av('mp', bf16)
            wpf_v = av('wpf', f32)
            wm_v = {mi: av(f'wm{mi}', f32) for mi in (1, 2)}
            bm_v = {mi: av(f'bm{mi}', f32) for mi in (1, 2)}
            msc_v = {(mi, sf, kind): av(f'{kind}_{sf}{mi}', f32)
                     for mi in (1, 2) for sf in 'sf'
                     for kind in ('scale', 'bias')}
            gmask_v = av('gmask', f32)

            id32 = cst.tile([32, 32], f32, tag='id32')
            make_identity(nc, id32[:])

            # ---------------- masif (both branches) -> two [64, B] f32 tiles
            for mi in (1, 2):
                frag = None
                for si, sf in enumerate('sf'):
                    toff = ((mi - 1) * 2 + si) * 6400
                    mv = arena2_t[:, toff:toff + 6400].bitcast(f32) \
                        .rearrange("p (c l) -> p c l", c=cfg.C)
                    red = smp.tile([32, cfg.LBS], f32, tag='masred')
                    nc.vector.tensor_reduce(
                        out=red[:], in_=mv.transpose([0, 2, 1]),
                        axis=mybir.AxisListType.X, op=OP.add)
                    act = smp.tile([32, cfg.LBS], f32, tag='masact')
                    nc.scalar.activation(
                        act[:], red[:], AF.Relu,
                        bias=msc_v[(mi, sf, 'bias')][:, 0:1],
                        scale=msc_v[(mi, sf, 'scale')][:, 0:1])
                    ws = smp.tile([32, cfg.WPB], f32, tag='masws')
                    nc.vector.tensor_reduce(
                        out=ws[:],
                        in_=act[:].rearrange("p (w l) -> p w l", l=cfg.LW),
                        axis=mybir.AxisListType.X, op=OP.add)
                    if frag is None:
                        frag = ws
                    else:
                        frag2 = smp.tile([32, cfg.WPB], f32, tag='masfrag')
                        nc.vector.tensor_add(out=frag2[:], in0=frag[:],
                                             in1=ws[:])
                        frag = frag2
                ps_t = smps.tile([cfg.WPB, 32], f32, space="PSUM", tag='sps')
                nc.tensor.transpose(out=ps_t[:], in_=frag[:], identity=id32[:])
                fragT = smp.tile([cfg.WPB, 32], f32, tag='masfragT')
                nc.scalar.activation(fragT[:], ps_t[:], AF.Identity)
                fragTc = fragT[:].rearrange("k (lb g) -> k lb g", g=cfg.GPB)
                m_ps = smps.tile([64, cfg.GPB], f32, space="PSUM", tag='sps')
                for lb in range(cfg.LB):
                    nc.tensor.matmul(
                        m_ps[:], lhsT=wm_v[mi][:, lb, :], rhs=fragTc[:, lb, :],
                        start=(lb == 0), stop=(lb == cfg.LB - 1))
                m_fm = smp.tile([64, cfg.GPB], f32, tag='masfm')
                nc.scalar.activation(m_fm[:], m_ps[:], AF.Identity,
                                     bias=bm_v[mi][:, 0:1])
                masm = cst.tile([64, cfg.B], f32, tag=f'masasm{mi}')
                nc.vector.tensor_tensor(
                    out=masm[:].rearrange("p (s g) -> p s g", g=cfg.GPB),
                    in0=m_fm[:, None, :].to_broadcast(
                        [64, N_CORES, cfg.GPB]),
                    in1=gmask_v.rearrange("p (s g) -> p s g", g=cfg.GPB),
                    op=OP.mult)
                nc.sync.dma_start(
                    out=out_t.ap()[(mi - 1) * 64:mi * 64, 32:64], in_=masm[:])

            # ---------------- GCN branch (this core's branch + quarter)
            xw_loc = drp.tile([cfg.QUART, cfg.F], fp8, tag='xwloc')

            # xw = x_quarter @ W (fp8 DoubleRow, x stationary)
            for nt in range(cfg.NT):
                xt_t = xtp.tile([P, cfg.KC, 512], fp8, tag='xt')
                nc.sync.dma_start(out=xt_t[:], in_=xT_d.ap()[nt])
                xw_t = xwsb.tile([P, 4, cfg.F], fp8, tag='xwsb')
                for sub in range(4):
                    for hf in range(2):
                        ps = xwps.tile([P, 512], f32, space="PSUM",
                                       tag='xwps')
                        for kpi in range(cfg.KP):
                            nc.tensor.matmul(
                                ps[:],
                                lhsT=xt_t[:, 2 * kpi:2 * kpi + 2,
                                          sub * P:(sub + 1) * P],
                                rhs=wg_v[:, 2 * kpi:2 * kpi + 2,
                                         hf * 512:(hf + 1) * 512],
                                start=(kpi == 0), stop=(kpi == cfg.KP - 1),
                                perf_mode=DR)
                        if (sub + hf) % 2 == 0:
                            nc.vector.tensor_scalar_mul(
                                xw_t[:, sub, hf * 512:(hf + 1) * 512],
                                ps[:], 1.0)
                        else:
                            nc.scalar.activation(
                                xw_t[:, sub, hf * 512:(hf + 1) * 512],
                                ps[:], AF.Identity)
                nc.sync.dma_start(
                    out=xw_loc[nt * 512:(nt + 1) * 512, :].rearrange(
                        "(s p) f -> p s f", p=P),
                    in_=xw_t[:])

            # AllGather xw across all 8 cores into ONE shared HBM buffer
            # (ranks 0-3 = branch 1 quarters -> rows 0:NPAD, ranks 4-7 =
            # branch 2 -> rows NPAD:2*NPAD). Shared output => each core
            # writes only its own 2.6MB slice, no ring traffic.
            nc.gpsimd.collective_compute(
                "AllGather", mybir.AluOpType.bypass,
                replica_groups=[[0, 1, 2, 3, 4, 5, 6, 7]],
                ins=[xw_loc[:]],
                outs=[xw_all.ap()])

            # scatter + transposed pool
            poolps = poolp.tile([P, cfg.KC, cfg.B], f32, space="PSUM",
                                tag='poolps')
            pi = 0
            psA = psB = None
            for g in range(n_call):
                scs_t = scsp.tile([P, 128 + GRP * BLK], u8, tag='scs')
                nc.sync.dma_start(out=scs_t[:], in_=scs_d.ap()[g])
                idx_v = scs_t[:, 0:128].bitcast(dt.int16)
                s_v = scs_t[:, 128:128 + GRP * BLK].bitcast(fp8) \
                    .rearrange("p (c d) -> p c d", c=GRP)
                gat_t = gatp.tile([P, GRP, cfg.F], fp8, tag='gat')
                nc.gpsimd.dma_gather(
                    out_ap=gat_t[:], in_ap=xw_all.ap(),
                    idxs_ap=idx_v,
                    num_idxs=GRP * P, num_idxs_reg=GRP * P,
                    elem_size=cfg.F, queue_num=g % 2)
                for i in range(PAIRS_PER_CALL):
                    j, st, sp = sched[pi]
                    if st:
                        psA = blkpsA.tile([P, 512], f32, space="PSUM",
                                          tag='blkA')
                        psB = blkpsB.tile([P, 512], f32, space="PSUM",
                                          tag='blkB')
                    nc.tensor.matmul(
                        psA[:],
                        lhsT=s_v[:, 2 * i:2 * i + 2, :],
                        rhs=gat_t[:, 2 * i:2 * i + 2, 0:512],
                        start=st, stop=sp, perf_mode=DR)
                    nc.tensor.matmul(
                        psB[:],
                        lhsT=s_v[:, 2 * i:2 * i + 2, :],
                        rhs=gat_t[:, 2 * i:2 * i + 2, 512:1024],
                        start=st, stop=sp, perf_mode=DR)
                    if sp:
                        h_t = hp.tile([P, cfg.F], bf16, tag='h')
                        nc.vector.tensor_add(out=h_t[:, 0:512], in0=psA[:],
                                             in1=bg_v[:, 0:512])
                        nc.vector.tensor_add(out=h_t[:, 512:1024], in0=psB[:],
                                             in1=bg_v[:, 512:1024])
                        nc.scalar.activation(h_t[:], h_t[:], AF.Lrelu,
                                             alpha=0.01)
                        for c in range(cfg.KC):
                            nc.tensor.matmul(
                                poolps[:, c, :],
                                lhsT=h_t[:, c * P:(c + 1) * P],
                                rhs=mp_v[:, j, :],
                                start=(j == 0), stop=(j == cfg.NBLK - 1),
                                skip_group_check=True)
                    pi += 1

            # x_pre partial: [128, B] = W_pf^T @ pooledT
            pooled_sb = smp.tile([P, cfg.KC, cfg.B], f32, tag='pooled')
            nc.vector.tensor_scalar_mul(pooled_sb[:], poolps[:], 1.0)
            xpre_ps = smps.tile([P, cfg.B], f32, space="PSUM", tag='sps')
            for c in range(cfg.KC):
                nc.tensor.matmul(xpre_ps[:], lhsT=wpf_v[:, c, :],
                                 rhs=pooled_sb[:, c, :],
                                 start=(c == 0), stop=(c == cfg.KC - 1))
            xpre_sb = smp.tile([P, cfg.B], f32, tag='xpresb')
            nc.vector.tensor_scalar_mul(xpre_sb[:], xpre_ps[:], 1.0)
            nc.sync.dma_start(out=out_t.ap()[:, 0:32], in_=xpre_sb[:])

    nc.compile()
    return nc


# ---------------------------------------------------------------- entry
_CACHE = {}


def _run(inputs, cfg, trace=False, tmpdir=None):
    from concourse import bass_utils
    meta, in_maps = _preprocess(inputs, cfg)
    key = (cfg.N, cfg.F, meta['npairs'], meta['kp'])
    if key not in _CACHE:
        _CACHE.clear()
        _CACHE[key] = _build(cfg, meta)
    nc = _CACHE[key]
    res = bass_utils.run_bass_kernel_spmd(
        nc, in_maps, core_ids=list(range(N_CORES)), trace=trace, tmpdir=tmpdir)
    outs = [np.asarray(res.results[i]['out'], np.float64)
            for i in range(N_CORES)]
    out = _host_head(inputs, cfg, outs)
    return out, res


def _lrelu(v):
    return np.where(v > 0, v, 0.01 * v)


def _host_head(inputs, cfg, outs):
    """Unshard: sum of per-core partials -> tiny dense head (host)."""
    f32 = np.float32
    xr1 = sum(outs[i][:, 0:32] for i in range(4))
    xr2 = sum(outs[i][:, 0:32] for i in range(4, 8))
    m1 = sum(outs[i][0:64, 32:64] for i in range(N_CORES))
    m2 = sum(outs[i][64:128, 32:64] for i in range(N_CORES))
    x1 = _lrelu(xr1 + np.asarray(inputs['b_pf1'], f32)[:, None])
    x2 = _lrelu(xr2 + np.asarray(inputs['b_pf2'], f32)[:, None])
    xcat = np.concatenate([x1, x2], 0)                       # [256, B]
    xc1 = _lrelu(np.asarray(inputs['W_fc1'], f32).T @ xcat
                 + np.asarray(inputs['b_fc1'], f32)[:, None])
    xc2 = _lrelu(np.asarray(inputs['W_fc2'], f32).T @ xc1
                 + np.asarray(inputs['b_fc2'], f32)[:, None])
    W_out = np.asarray(inputs['W_out'], f32)
    z = (W_out[0:64].T @ xc2 + W_out[64:128].T @ m1 + W_out[128:192].T @ m2
         + np.asarray(inputs['b_out'], f32)[:, None])
    return (1.0 / (1.0 + np.exp(-z))).T.astype(f32)         # [B, 1]


def kernel(**inputs) -> np.ndarray:
    cfg = _Cfg()
    out, _ = _run(inputs, cfg)
    return out


# revision 16
# speedup vs baseline: 1.2963x; 1.1263x over previous
"""Trainium2 Bass kernel for nn_GCNN_87668872446200 (v3: branch-split + AllGather).

Two GCNConv+pool protein branches + two masif conv branches + dense head,
distributed over 8 NeuronCores as 2 branch-groups x 4 dest-node quarters.

Cores 0-3 handle protein branch 1 (quarters 0-3), cores 4-7 branch 2.
Per core (full 1024-dim features on the heavy paths, fp8):
  - xw = x_quarter @ W via fp8 DoubleRow matmuls for ONLY this core's 2560
    nodes -> local DRAM [2560, 1024] fp8 (no redundant compute)
  - ONE AllGather per 4-core branch group -> xw_full [10240, 1024] fp8
  - dma_gather pulls full 1KB source rows for this core's edge quarter
    (half the descriptors vs 512B rows, better HBM efficiency)
  - scatter-add as fp8 DoubleRow PE matmuls: S[256 edges, 128 dests]
    (host-built, norm-scaled, degree-balanced dest blocks) x gathered
    [256, 1024] accumulated in PSUM (2 x 512 halves)
  - h = lrelu(psum + bias) [128, 1024] bf16; transposed mean-pool via PE
    (pooledT [1024, 32]) folding 1/cnt
  - x_pre = W_pf^T @ pooledT partial [128, 32] (pre-activation, linear ->
    summable across the 4 quarter cores on host)
  - masif branch: 4 graphs/core (all cores, same as before)
  - out [128, 64]: cols 0:32 xpre partial, cols 32:64 masif partials
Host: sum partials per branch group, run the tiny dense head.

All 8 cores run ONE identical program; per-core variation is in input data.
"""
import numpy as np

# ---------------------------------------------------------------- constants
N_CORES = 8
N_QUART = 4       # dest-node quarters per branch group
P = 128
BLK = 128         # dest nodes per scatter block (S width)
PAIR_E = 256      # edges per DoubleRow matmul (2 chunks of 128)
GRP = 8           # chunks per dma_gather call (1024 idxs per call)
PAIRS_PER_CALL = GRP // 2

# problem sizes (hardcoded per spec)
N_NODES, N_EDGES, F_DIM, B_GRAPHS, L_MAS, C_MAS = 10000, 80000, 1024, 32, 800, 16


def _fp8():
    import ml_dtypes
    return ml_dtypes.float8_e4m3fn


def _bf16():
    import ml_dtypes
    return ml_dtypes.bfloat16


class _Cfg:
    def __init__(self, n=N_NODES, e=N_EDGES, f=F_DIM, b=B_GRAPHS,
                 l=L_MAS, c=C_MAS):
        assert f % 512 == 0 and b == 32 and l % 80 == 0 and c % 2 == 0
        self.N, self.E, self.F, self.B, self.L, self.C = n, e, f, b, l, c
        self.NPAD = ((n + 2047) // 2048) * 2048
        self.QUART = self.NPAD // N_QUART      # nodes per quarter (2560)
        assert self.QUART % 512 == 0
        self.NT = self.QUART // 512            # local node tiles (5)
        self.NBLK = self.QUART // BLK          # dest blocks per quarter (20)
        self.KC = f // P                       # k-chunks of contraction (8)
        self.KP = self.KC // 2                 # k-pairs (DoubleRow) (4)
        self.GPB = b // N_CORES                # graphs per core for masif
        self.LW = l // 80                      # avg-pool window (10)
        self.LB = 8                            # l-blocks for masif layout
        self.LBS = l // self.LB                # l-block size (100)
        assert self.LBS % self.LW == 0
        self.WPB = self.LBS // self.LW         # windows per l-block (10)


# ------------------------------------------------------------- arena layout
# (name, rows, dtype-key, shape) -- shared by host packer and kernel views
def _arena_layout(cfg):
    return [
        ('wg', 128, 'fp8', (cfg.KC, cfg.F)),
        ('bg', 128, 'f32', (cfg.F,)),
        ('mp', 128, 'bf16', (cfg.NBLK, cfg.B)),
        ('wpf', 128, 'f32', (cfg.KC, 128)),
        ('wm1', 10, 'f32', (8, 64)),
        ('wm2', 10, 'f32', (8, 64)),
        ('bm1', 64, 'f32', (1,)),
        ('bm2', 64, 'f32', (1,)),
        ('scale_s1', 32, 'f32', (1,)),
        ('bias_s1', 32, 'f32', (1,)),
        ('scale_f1', 32, 'f32', (1,)),
        ('bias_f1', 32, 'f32', (1,)),
        ('scale_s2', 32, 'f32', (1,)),
        ('bias_s2', 32, 'f32', (1,)),
        ('scale_f2', 32, 'f32', (1,)),
        ('bias_f2', 32, 'f32', (1,)),
        ('gmask', 64, 'f32', (cfg.B,)),
    ]


def _dt_size(key):
    return {'f32': 4, 'bf16': 2, 'fp8': 1}[key]


def _arena_offsets(cfg):
    off, out = 0, {}
    for name, rows, key, shape in _arena_layout(cfg):
        nb = int(np.prod(shape)) * _dt_size(key)
        out[name] = (off, rows, key, shape, nb)
        off += (nb + 63) // 64 * 64
    return out, off


# ---------------------------------------------------------------- host prep
def _edge_plan(cfg, edge_indices):
    """Shared block-pair profile (kp) + per-shard scatter plans.

    Returns (kp, sched, npairs, shards) where shards[(br, qu)] =
    (srcs [nchunk,128] int16, smat [nchunk,128,128] fp8, dest_of) and
    dest_of[(br, qu)] maps block,offset -> global dest node (for mp).
    """
    fp8 = _fp8()
    per_shard = {}
    for br in (1, 2):
        ei = edge_indices[br]
        row = np.asarray(ei[0]).astype(np.int64)
        col = np.asarray(ei[1]).astype(np.int64)
        loops = np.arange(cfg.N, dtype=np.int64)
        rows = np.concatenate([row, loops])
        cols = np.concatenate([col, loops])
        deg = np.bincount(cols, minlength=cfg.N).astype(np.float64)
        dinv = 1.0 / np.sqrt(deg)
        norm = (dinv[rows] * dinv[cols]).astype(np.float32)
        for qu in range(N_QUART):
            lo = qu * cfg.QUART
            sel = (cols >= lo) & (cols < lo + cfg.QUART)
            per_shard[(br, qu)] = (rows[sel], cols[sel] - lo, norm[sel])

    e_max = max(len(r) for r, _, _ in per_shard.values())
    npairs = -(-int(e_max * 1.02) // PAIR_E)
    npairs = -(-npairs // PAIRS_PER_CALL) * PAIRS_PER_CALL

    # find a feasible shared kp profile (block capacities) for all shards
    while True:
        base, extra = npairs // cfg.NBLK, npairs % cfg.NBLK
        kp = np.full(cfg.NBLK, base, np.int64)
        kp[:extra] += 1
        caps = kp * PAIR_E
        ok = True
        assigns = {}
        for key, (r, c, w) in per_shard.items():
            cnt = np.bincount(c, minlength=cfg.QUART)
            a = _pack_blocks(cfg, cnt, caps)
            if a is None:
                ok = False
                break
            assigns[key] = a
        if ok:
            break
        npairs += PAIRS_PER_CALL

    sched = []
    for j in range(cfg.NBLK):
        for t in range(kp[j]):
            sched.append((j, t == 0, t == int(kp[j]) - 1))
    assert len(sched) == npairs
    base_ch = np.zeros(cfg.NBLK, np.int64)
    base_ch[1:] = np.cumsum(2 * kp)[:-1]
    nchunk = 2 * npairs

    shards = {}
    for key, (r, c, w) in per_shard.items():
        blk_of, off_of = assigns[key]          # per local dest
        order = np.lexsort((off_of[c], blk_of[c]))
        r, c, w = r[order], c[order], w[order]
        b = blk_of[c]
        starts = np.searchsorted(b, np.arange(cfg.NBLK), 'left')
        o = np.arange(len(r)) - starts[b]
        ch = base_ch[b] + o // P
        sl = o % P
        srcs = np.zeros((nchunk, P), np.int16)
        smat = np.zeros((nchunk, P, BLK), fp8)
        srcs[ch, sl] = r.astype(np.int16)
        smat[ch, sl, off_of[c]] = w.astype(fp8)
        shards[key] = (srcs, smat, (blk_of, off_of))
    return kp, sched, npairs, shards


def _pack_blocks(cfg, cnt, caps):
    """Assign QUART dests into NBLK blocks of exactly BLK dests so block
    edge-counts fit caps. Greedy: heaviest dest -> block with most slack."""
    nb = cfg.NBLK
    order = np.argsort(-cnt, kind='stable')
    load = np.zeros(nb, np.int64)
    nmem = np.zeros(nb, np.int64)
    blk_of = np.zeros(cfg.QUART, np.int64)
    off_of = np.zeros(cfg.QUART, np.int64)
    for d in order:
        slack = caps - load - cnt[d]
        slack[nmem >= BLK] = np.iinfo(np.int64).min
        j = int(np.argmax(slack))
        if slack[j] < 0:
            return None
        blk_of[d] = j
        off_of[d] = nmem[j]
        load[j] += cnt[d]
        nmem[j] += 1
    assert (nmem == BLK).all()
    return blk_of, off_of


def _wrap_idxs(srcs):
    """[C, 128] int16 -> wrapped [128, C*8] (idx j at [j%16 + 16*rep, j//16])."""
    flat = srcs.reshape(-1)
    w = flat.reshape(-1, 16).T                # [16, C*8]
    return np.ascontiguousarray(np.tile(w, (8, 1)).astype(np.int16))


def _build_scs(srcs, smat):
    """Combine wrapped idxs + grouped smat into one [calls, 128, 1152] u8."""
    nchunk = srcs.shape[0]
    calls = nchunk // GRP
    idxw = _wrap_idxs(srcs)                   # [128, nchunk*8] int16
    scs = np.zeros((calls, P, 128 + GRP * BLK), np.uint8)
    idxu = idxw.view(np.uint8).reshape(P, calls, 128).transpose(1, 0, 2)
    scs[:, :, 0:128] = idxu
    smu = smat.view(np.uint8).reshape(calls, GRP, P, BLK)
    scs[:, :, 128:] = smu.transpose(0, 2, 1, 3).reshape(calls, P, GRP * BLK)
    return scs


def _mpool(cfg, batch, qu, assign):
    """[128, NBLK, B] bf16 folding 1/cnt, zero rows for pad nodes."""
    batch = np.asarray(batch).astype(np.int64)
    cnt = np.bincount(batch, minlength=cfg.B).astype(np.float64)
    cinv = (1.0 / np.maximum(cnt, 1.0)).astype(np.float32)
    blk_of, off_of = assign
    m = np.zeros((P, cfg.NBLK, cfg.B), np.float32)
    lo = qu * cfg.QUART
    hi = min(lo + cfg.QUART, cfg.N)
    if hi > lo:
        nodes = np.arange(lo, hi)
        rel = nodes - lo
        m[off_of[rel], blk_of[rel], batch[nodes]] = cinv[batch[nodes]]
    return m.astype(_bf16())


def _xquant(cfg, x):
    """x [N, F] f32 -> padded [NPAD, F] fp8 (gather source rows)."""
    fp8 = _fp8()
    x8 = np.zeros((cfg.NPAD, cfg.F), fp8)
    x8[:cfg.N] = np.asarray(x, np.float32).astype(fp8)
    return x8


def _pack_arena(cfg, arrays):
    offs, total = _arena_offsets(cfg)
    ab = (total + 63) // 64 * 64
    arena = np.zeros((P, ab), np.uint8)
    for name, (off, rows, key, shape, nb) in offs.items():
        a = arrays[name]
        assert a.shape == (rows,) + tuple(shape), (name, a.shape, rows, shape)
        npdt = {'f32': np.float32, 'bf16': _bf16(), 'fp8': _fp8()}[key]
        flat = np.ascontiguousarray(a.astype(npdt)).view(np.uint8).reshape(rows, nb)
        arena[:rows, off:off + nb] = flat
    return arena


def _preprocess(inputs, cfg):
    fp8 = _fp8()
    kp, sched, npairs, shards = _edge_plan(
        cfg, {1: inputs['pro1_edge_index'], 2: inputs['pro2_edge_index']})
    meta = {'kp': tuple(int(v) for v in kp), 'sched': sched, 'npairs': npairs}

    def f32(v):
        return np.asarray(v, np.float32)

    xg = {br: _xquant(cfg, inputs[f'pro{br}_x']) for br in (1, 2)}
    in_maps = []
    for core in range(N_CORES):
        br, qu = core // N_QUART + 1, core % N_QUART
        ar = {}
        Wg = f32(inputs[f'W_g{br}'])
        ar['wg'] = np.ascontiguousarray(
            Wg.reshape(cfg.KC, P, cfg.F).transpose(1, 0, 2)).astype(fp8)
        ar['bg'] = np.tile(f32(inputs[f'b_g{br}'])[None, :], (P, 1))
        srcs, smat, assign = shards[(br, qu)]
        ar['mp'] = _mpool(cfg, inputs[f'pro{br}_batch'], qu, assign)
        Wpf = f32(inputs[f'W_pf{br}'])
        ar['wpf'] = np.ascontiguousarray(
            Wpf.reshape(cfg.KC, P, P).transpose(1, 0, 2))
        for mi in (1, 2):
            ar[f'wm{mi}'] = np.ascontiguousarray(
                (f32(inputs[f'W_m{mi}']) / (2.0 * cfg.LW))
                .reshape(8, 10, 64).transpose(1, 0, 2))
            ar[f'bm{mi}'] = f32(inputs[f'b_m{mi}']).reshape(64, 1)
            for sf, pre in (('s', 'cs'), ('f', 'cf')):
                w = float(np.asarray(inputs[f'{pre}{mi}_w'])[0])
                b = float(np.asarray(inputs[f'{pre}{mi}_b'])[0])
                ar[f'scale_{sf}{mi}'] = np.full((32, 1), w / cfg.C, np.float32)
                ar[f'bias_{sf}{mi}'] = np.full((32, 1), b, np.float32)
        gm = np.zeros((64, cfg.B), np.float32)
        gm[:, core * cfg.GPB:(core + 1) * cfg.GPB] = 1.0
        ar['gmask'] = gm

        arena = _pack_arena(cfg, ar)

        # masif arena2: 4 tensors x [32, 16, 100] f32 = 4 x 6400B per row
        a2 = np.zeros((32, 4 * 6400), np.uint8)
        for ti, name in enumerate(['mas1_straight', 'mas1_flipped',
                                   'mas2_straight', 'mas2_flipped']):
            a = f32(inputs[name])[core * cfg.GPB:(core + 1) * cfg.GPB]
            blk = a.reshape(cfg.GPB, cfg.C, cfg.LB, cfg.LBS) \
                   .transpose(2, 0, 1, 3).reshape(32, cfg.C * cfg.LBS)
            a2[:, ti * 6400:(ti + 1) * 6400] = \
                np.ascontiguousarray(blk).view(np.uint8)

        m = {'arena': arena, 'arena2': a2,
             'xg': xg[br],
             'scs': _build_scs(srcs, smat)}
        in_maps.append(m)
    return meta, in_maps


# ---------------------------------------------------------------- program
def _build(cfg, meta):
    import concourse.bass as bass
    import concourse.bacc as bacc
    import concourse.mybir as mybir
    import concourse.tile as tile
    from concourse.masks import make_identity

    dt = mybir.dt
    fp8 = dt.float8e4
    bf16 = dt.bfloat16
    f32 = dt.float32
    u8 = dt.uint8
    AF = mybir.ActivationFunctionType
    OP = mybir.AluOpType
    DR = mybir.MatmulPerfMode.DoubleRow

    nc = bacc.Bacc("TRN2", target_bir_lowering=False, debug=False,
                   enable_asserts=False, num_devices=N_CORES,
                   num_swdge_queues=2)

    offs, total = _arena_offsets(cfg)
    AB = (total + 63) // 64 * 64
    npairs = meta['npairs']
    sched = meta['sched']
    n_call = npairs // PAIRS_PER_CALL

    arena_d = nc.dram_tensor('arena', [P, AB], u8, kind="ExternalInput")
    arena2_d = nc.dram_tensor('arena2', [32, 4 * 6400], u8, kind="ExternalInput")
    xg_d = nc.dram_tensor('xg', [cfg.NPAD, cfg.F], fp8, kind="ExternalInput")
    scs_d = nc.dram_tensor('scs', [n_call, P, 128 + GRP * BLK], u8,
                           kind="ExternalInput")
    out_t = nc.dram_tensor('out', [P, 64], f32, kind="ExternalOutput")

    with tile.TileContext(nc) as tc:
        with tc.tile_pool(name="const", bufs=1) as cst, \
             tc.tile_pool(name="scs", bufs=12) as scsp, \
             tc.tile_pool(name="gat", bufs=8) as gatp, \
             tc.tile_pool(name="aggps", bufs=2, space="PSUM") as aggpsp, \
             tc.tile_pool(name="aggsb", bufs=2) as aggsbp, \
             tc.tile_pool(name="hps", bufs=1, space="PSUM") as hpsp, \
             tc.tile_pool(name="hb", bufs=3) as hp, \
             tc.tile_pool(name="poolps", bufs=1, space="PSUM") as poolp, \
             tc.tile_pool(name="small", bufs=2) as smp, \
             tc.tile_pool(name="smallps", bufs=1, space="PSUM") as smps:

            # ---------------- constant arena (one DMA each)
            arena_t = cst.tile([P, AB], u8, tag='arena')
            nc.sync.dma_start(out=arena_t[:], in_=arena_d.ap())
            arena2_t = cst.tile([32, 4 * 6400], u8, tag='arena2')
            nc.sync.dma_start(out=arena2_t[:], in_=arena2_d.ap())

            def av(name, dtype):
                off, rows, key, shape, nb = offs[name]
                v = arena_t[0:rows, off:off + nb].bitcast(dtype)
                if len(shape) == 2:
                    v = v.rearrange("p (a b) -> p a b", a=shape[0])
                return v

            wg_v = av('wg', fp8)
            bg_v = av('bg', f32)
            mp_v = av('mp', bf16)
            wpf_v = av('wpf', f32)
            wm_v = {mi: av(f'wm{mi}', f32) for mi in (1, 2)}
            bm_v = {mi: av(f'bm{mi}', f32) for mi in (1, 2)}
            msc_v = {(mi, sf, kind): av(f'{kind}_{sf}{mi}', f32)
                     for mi in (1, 2) for sf in 'sf'
                     for kind in ('scale', 'bias')}
            gmask_v = av('gmask', f32)

            id32 = cst.tile([32, 32], f32, tag='id32')
            make_identity(nc, id32[:])

            # ---------------- masif (both branches) -> two [64, B] f32 tiles
            for mi in (1, 2):
                frag = None
                for si, sf in enumerate('sf'):
                    toff = ((mi - 1) * 2 + si) * 6400
                    mv = arena2_t[:, toff:toff + 6400].bitcast(f32) \
                        .rearrange("p (c l) -> p c l", c=cfg.C)
                    red = smp.tile([32, cfg.LBS], f32, tag='masred')
                    nc.vector.tensor_reduce(
                        out=red[:], in_=mv.transpose([0, 2, 1]),
                        axis=mybir.AxisListType.X, op=OP.add)
                    act = smp.tile([32, cfg.LBS], f32, tag='masact')
                    nc.scalar.activation(
                        act[:], red[:], AF.Relu,
                        bias=msc_v[(mi, sf, 'bias')][:, 0:1],
                        scale=msc_v[(mi, sf, 'scale')][:, 0:1])
                    ws = smp.tile([32, cfg.WPB], f32, tag='masws')
                    nc.vector.tensor_reduce(
                        out=ws[:],
                        in_=act[:].rearrange("p (w l) -> p w l", l=cfg.LW),
                        axis=mybir.AxisListType.X, op=OP.add)
                    if frag is None:
                        frag = ws
                    else:
                        frag2 = smp.tile([32, cfg.WPB], f32, tag='masfrag')
                        nc.vector.tensor_add(out=frag2[:], in0=frag[:],
                                             in1=ws[:])
                        frag = frag2
                ps_t = smps.tile([cfg.WPB, 32], f32, space="PSUM", tag='sps')
                nc.tensor.transpose(out=ps_t[:], in_=frag[:], identity=id32[:])
                fragT = smp.tile([cfg.WPB, 32], f32, tag='masfragT')
                nc.scalar.activation(fragT[:], ps_t[:], AF.Identity)
                fragTc = fragT[:].rearrange("k (lb g) -> k lb g", g=cfg.GPB)
                m_ps = smps.tile([64, cfg.GPB], f32, space="PSUM", tag='sps')
                for lb in range(cfg.LB):
                    nc.tensor.matmul(
                        m_ps[:], lhsT=wm_v[mi][:, lb, :], rhs=fragTc[:, lb, :],
                        start=(lb == 0), stop=(lb == cfg.LB - 1))
                m_fm = smp.tile([64, cfg.GPB], f32, tag='masfm')
                nc.scalar.activation(m_fm[:], m_ps[:], AF.Identity,
                                     bias=bm_v[mi][:, 0:1])
                masm = cst.tile([64, cfg.B], f32, tag=f'masasm{mi}')
                nc.vector.tensor_tensor(
                    out=masm[:].rearrange("p (s g) -> p s g", g=cfg.GPB),
                    in0=m_fm[:, None, :].to_broadcast(
                        [64, N_CORES, cfg.GPB]),
                    in1=gmask_v.rearrange("p (s g) -> p s g", g=cfg.GPB),
                    op=OP.mult)
                nc.sync.dma_start(
                    out=out_t.ap()[(mi - 1) * 64:mi * 64, 32:64], in_=masm[:])

            # ---------------- GCN branch (this core's branch + quarter)
            # aggregate-first: aggT[f, d] = sum_e x[src_e, f] * S[e, d]
            # gathered straight from the x input tensor (no producer dep),
            # then h = lrelu(aggT^T @ W + b) per 128-dest block.
            poolps = poolp.tile([P, cfg.KC, cfg.B], f32, space="PSUM",
                                tag='poolps')
            pi = 0
            agg_ps = None
            for g in range(n_call):
                scs_t = scsp.tile([P, 128 + GRP * BLK], u8, tag='scs')
                nc.sync.dma_start(out=scs_t[:], in_=scs_d.ap()[g])
                idx_v = scs_t[:, 0:128].bitcast(dt.int16)
                s_v = scs_t[:, 128:128 + GRP * BLK].bitcast(fp8) \
                    .rearrange("p (c d) -> p c d", c=GRP)
                gat_t = gatp.tile([P, GRP, cfg.F], fp8, tag='gat')
                nc.gpsimd.dma_gather(
                    out_ap=gat_t[:], in_ap=xg_d.ap(),
                    idxs_ap=idx_v,
                    num_idxs=GRP * P, num_idxs_reg=GRP * P,
                    elem_size=cfg.F, queue_num=g % 2)
                for i in range(PAIRS_PER_CALL):
                    j, st, sp = sched[pi]
                    if st:
                        agg_ps = aggpsp.tile([P, cfg.KC, BLK], f32,
                                             space="PSUM", tag='aggps')
                    for c in range(cfg.KC):
                        nc.tensor.matmul(
                            agg_ps[:, c, :],
                            lhsT=gat_t[:, 2 * i:2 * i + 2,
                                       c * P:(c + 1) * P],
                            rhs=s_v[:, 2 * i:2 * i + 2, :],
                            start=st, stop=sp, perf_mode=DR,
                            skip_group_check=True)
                    if sp:
                        agg_sb = aggsbp.tile([P, cfg.KC, BLK], fp8,
                                             tag='aggsb')
                        if j % 2 == 0:
                            nc.vector.tensor_scalar_mul(
                                agg_sb[:], agg_ps[:], 1.0)
                        else:
                            nc.scalar.activation(
                                agg_sb[:], agg_ps[:], AF.Identity)
                        h_ps = hpsp.tile([P, cfg.F], f32, space="PSUM",
                                         tag='hps')
                        for c in range(cfg.KP):
                            for hf in range(2):
                                nc.tensor.matmul(
                                    h_ps[:, hf * 512:(hf + 1) * 512],
                                    lhsT=agg_sb[:, 2 * c:2 * c + 2, :],
                                    rhs=wg_v[:, 2 * c:2 * c + 2,
                                             hf * 512:(hf + 1) * 512],
                                    start=(c == 0), stop=(c == cfg.KP - 1),
                                    perf_mode=DR, skip_group_check=True)
                        h_t = hp.tile([P, cfg.F], bf16, tag='h')
                        nc.vector.tensor_add(out=h_t[:], in0=h_ps[:],
                                             in1=bg_v[:])
                        nc.scalar.activation(h_t[:], h_t[:], AF.Lrelu,
                                             alpha=0.01)
                        for c in range(cfg.KC):
                            nc.tensor.matmul(
                                poolps[:, c, :],
                                lhsT=h_t[:, c * P:(c + 1) * P],
                                rhs=mp_v[:, j, :],
                                start=(j == 0), stop=(j == cfg.NBLK - 1),
                                skip_group_check=True)
                    pi += 1

            # x_pre partial: [128, B] = W_pf^T @ pooledT
            pooled_sb = smp.tile([P, cfg.KC, cfg.B], f32, tag='pooled')
            nc.vector.tensor_scalar_mul(pooled_sb[:], poolps[:], 1.0)
            xpre_ps = smps.tile([P, cfg.B], f32, space="PSUM", tag='sps')
            for c in range(cfg.KC):
                nc.tensor.matmul(xpre_ps[:], lhsT=wpf_v[:, c, :],
                                 rhs=pooled_sb[:, c, :],
                                 start=(c == 0), stop=(c == cfg.KC - 1))
            xpre_sb = smp.tile([P, cfg.B], f32, tag='xpresb')
            nc.vector.tensor_scalar_mul(xpre_sb[:], xpre_ps[:], 1.0)
            nc.sync.dma_start(out=out_t.ap()[:, 0:32], in_=xpre_sb[:])

    nc.compile()
    return nc


# ---------------------------------------------------------------- entry
_CACHE = {}


def _run(inputs, cfg, trace=False, tmpdir=None):
    from concourse import bass_utils
    meta, in_maps = _preprocess(inputs, cfg)
    key = (cfg.N, cfg.F, meta['npairs'], meta['kp'])
    if key not in _CACHE:
        _CACHE.clear()
        _CACHE[key] = _build(cfg, meta)
    nc = _CACHE[key]
    res = bass_utils.run_bass_kernel_spmd(
        nc, in_maps, core_ids=list(range(N_CORES)), trace=trace, tmpdir=tmpdir)
    outs = [np.asarray(res.results[i]['out'], np.float64)
            for i in range(N_CORES)]
    out = _host_head(inputs, cfg, outs)
    return out, res


def _lrelu(v):
    return np.where(v > 0, v, 0.01 * v)


def _host_head(inputs, cfg, outs):
    """Unshard: sum of per-core partials -> tiny dense head (host)."""
    f32 = np.float32
    xr1 = sum(outs[i][:, 0:32] for i in range(4))
    xr2 = sum(outs[i][:, 0:32] for i in range(4, 8))
    m1 = sum(outs[i][0:64, 32:64] for i in range(N_CORES))
    m2 = sum(outs[i][64:128, 32:64] for i in range(N_CORES))
    x1 = _lrelu(xr1 + np.asarray(inputs['b_pf1'], f32)[:, None])
    x2 = _lrelu(xr2 + np.asarray(inputs['b_pf2'], f32)[:, None])
    xcat = np.concatenate([x1, x2], 0)                       # [256, B]
    xc1 = _lrelu(np.asarray(inputs['W_fc1'], f32).T @ xcat
                 + np.asarray(inputs['b_fc1'], f32)[:, None])
    xc2 = _lrelu(np.asarray(inputs['W_fc2'], f32).T @ xc1
                 + np.asarray(inputs['b_fc2'], f32)[:, None])
    W_out = np.asarray(inputs['W_out'], f32)
    z = (W_out[0:64].T @ xc2 + W_out[64:128].T @ m1 + W_out[128:192].T @ m2
         + np.asarray(inputs['b_out'], f32)[:, None])
    return (1.0 / (1.0 + np.exp(-z))).T.astype(f32)         # [B, 1]


def kernel(**inputs) -> np.ndarray:
    cfg = _Cfg()
    out, _ = _run(inputs, cfg)
    return out


# revision 23
# speedup vs baseline: 1.6362x; 1.2622x over previous
"""Trainium2 Bass kernel for nn_GCNN_87668872446200 (v3: branch-split + AllGather).

Two GCNConv+pool protein branches + two masif conv branches + dense head,
distributed over 8 NeuronCores as 2 branch-groups x 4 dest-node quarters.

Cores 0-3 handle protein branch 1 (quarters 0-3), cores 4-7 branch 2.
Per core (full 1024-dim features on the heavy paths, fp8):
  - xw = x_quarter @ W via fp8 DoubleRow matmuls for ONLY this core's 2560
    nodes -> local DRAM [2560, 1024] fp8 (no redundant compute)
  - ONE AllGather per 4-core branch group -> xw_full [10240, 1024] fp8
  - dma_gather pulls full 1KB source rows for this core's edge quarter
    (half the descriptors vs 512B rows, better HBM efficiency)
  - scatter-add as fp8 DoubleRow PE matmuls: S[256 edges, 128 dests]
    (host-built, norm-scaled, degree-balanced dest blocks) x gathered
    [256, 1024] accumulated in PSUM (2 x 512 halves)
  - h = lrelu(psum + bias) [128, 1024] bf16; transposed mean-pool via PE
    (pooledT [1024, 32]) folding 1/cnt
  - x_pre = W_pf^T @ pooledT partial [128, 32] (pre-activation, linear ->
    summable across the 4 quarter cores on host)
  - masif branch: 4 graphs/core (all cores, same as before)
  - out [128, 64]: cols 0:32 xpre partial, cols 32:64 masif partials
Host: sum partials per branch group, run the tiny dense head.

All 8 cores run ONE identical program; per-core variation is in input data.
"""
import numpy as np

# ---------------------------------------------------------------- constants
N_CORES = 8
N_QUART = 4       # dest-node quarters per branch group
P = 128
BLK = 128         # dest nodes per scatter block (S width)
PAIR_E = 256      # edges per DoubleRow matmul (2 chunks of 128)
GRP = 8           # chunks per dma_gather call (1024 idxs per call)
PAIRS_PER_CALL = GRP // 2

# problem sizes (hardcoded per spec)
N_NODES, N_EDGES, F_DIM, B_GRAPHS, L_MAS, C_MAS = 10000, 80000, 1024, 32, 800, 16


def _fp8():
    import ml_dtypes
    return ml_dtypes.float8_e4m3fn


def _bf16():
    import ml_dtypes
    return ml_dtypes.bfloat16


class _Cfg:
    def __init__(self, n=N_NODES, e=N_EDGES, f=F_DIM, b=B_GRAPHS,
                 l=L_MAS, c=C_MAS):
        assert f % 512 == 0 and b == 32 and l % 80 == 0 and c % 2 == 0
        self.N, self.E, self.F, self.B, self.L, self.C = n, e, f, b, l, c
        self.NPAD = ((n + 2047) // 2048) * 2048
        self.QUART = self.NPAD // N_QUART      # nodes per quarter (2560)
        assert self.QUART % 512 == 0
        self.NT = self.QUART // 512            # local node tiles (5)
        self.NBLK = self.QUART // BLK          # dest blocks per quarter (20)
        self.KC = f // P                       # k-chunks of contraction (8)
        self.KP = self.KC // 2                 # k-pairs (DoubleRow) (4)
        self.GPB = b // N_CORES                # graphs per core for masif
        self.LW = l // 80                      # avg-pool window (10)
        self.LB = 8                            # l-blocks for masif layout
        self.LBS = l // self.LB                # l-block size (100)
        assert self.LBS % self.LW == 0
        self.WPB = self.LBS // self.LW         # windows per l-block (10)


# ------------------------------------------------------------- arena layout
# (name, rows, dtype-key, shape) -- shared by host packer and kernel views
def _arena_layout(cfg):
    return [
        ('wg', 128, 'fp8', (cfg.KC, cfg.F)),
        ('bg', 128, 'f32', (cfg.F,)),
        ('mp', 128, 'bf16', (cfg.NBLK, cfg.B)),
        ('wpf', 128, 'f32', (cfg.KC, 128)),
        ('wm1', 10, 'f32', (8, 64)),
        ('wm2', 10, 'f32', (8, 64)),
        ('bm1', 64, 'f32', (1,)),
        ('bm2', 64, 'f32', (1,)),
        ('scale_s1', 32, 'f32', (1,)),
        ('bias_s1', 32, 'f32', (1,)),
        ('scale_f1', 32, 'f32', (1,)),
        ('bias_f1', 32, 'f32', (1,)),
        ('scale_s2', 32, 'f32', (1,)),
        ('bias_s2', 32, 'f32', (1,)),
        ('scale_f2', 32, 'f32', (1,)),
        ('bias_f2', 32, 'f32', (1,)),
        ('gmask', 64, 'f32', (cfg.B,)),
    ]


def _dt_size(key):
    return {'f32': 4, 'bf16': 2, 'fp8': 1}[key]


def _arena_offsets(cfg):
    off, out = 0, {}
    for name, rows, key, shape in _arena_layout(cfg):
        nb = int(np.prod(shape)) * _dt_size(key)
        out[name] = (off, rows, key, shape, nb)
        off += (nb + 63) // 64 * 64
    return out, off


# ---------------------------------------------------------------- host prep
def _edge_plan(cfg, edge_indices):
    """Shared block-pair profile (kp) + per-shard scatter plans.

    Returns (kp, sched, npairs, shards) where shards[(br, qu)] =
    (srcs [nchunk,128] int16, smat [nchunk,128,128] fp8, dest_of) and
    dest_of[(br, qu)] maps block,offset -> global dest node (for mp).
    """
    fp8 = _fp8()
    per_shard = {}
    for br in (1, 2):
        ei = edge_indices[br]
        row = np.asarray(ei[0]).astype(np.int64)
        col = np.asarray(ei[1]).astype(np.int64)
        loops = np.arange(cfg.N, dtype=np.int64)
        rows = np.concatenate([row, loops])
        cols = np.concatenate([col, loops])
        deg = np.bincount(cols, minlength=cfg.N).astype(np.float64)
        dinv = 1.0 / np.sqrt(deg)
        norm = (dinv[rows] * dinv[cols]).astype(np.float32)
        for qu in range(N_QUART):
            lo = qu * cfg.QUART
            sel = (cols >= lo) & (cols < lo + cfg.QUART)
            per_shard[(br, qu)] = (rows[sel], cols[sel] - lo, norm[sel])

    e_max = max(len(r) for r, _, _ in per_shard.values())
    npairs = -(-int(e_max * 1.02) // PAIR_E)
    npairs = -(-npairs // PAIRS_PER_CALL) * PAIRS_PER_CALL

    # find a feasible shared kp profile (block capacities) for all shards
    while True:
        base, extra = npairs // cfg.NBLK, npairs % cfg.NBLK
        kp = np.full(cfg.NBLK, base, np.int64)
        kp[:extra] += 1
        caps = kp * PAIR_E
        ok = True
        assigns = {}
        for key, (r, c, w) in per_shard.items():
            cnt = np.bincount(c, minlength=cfg.QUART)
            a = _pack_blocks(cfg, cnt, caps)
            if a is None:
                ok = False
                break
            assigns[key] = a
        if ok:
            break
        npairs += PAIRS_PER_CALL

    sched = []
    for j in range(cfg.NBLK):
        for t in range(kp[j]):
            sched.append((j, t == 0, t == int(kp[j]) - 1))
    assert len(sched) == npairs
    base_ch = np.zeros(cfg.NBLK, np.int64)
    base_ch[1:] = np.cumsum(2 * kp)[:-1]
    nchunk = 2 * npairs

    shards = {}
    for key, (r, c, w) in per_shard.items():
        blk_of, off_of = assigns[key]          # per local dest
        order = np.lexsort((off_of[c], blk_of[c]))
        r, c, w = r[order], c[order], w[order]
        b = blk_of[c]
        starts = np.searchsorted(b, np.arange(cfg.NBLK), 'left')
        o = np.arange(len(r)) - starts[b]
        ch = base_ch[b] + o // P
        sl = o % P
        srcs = np.zeros((nchunk, P), np.int16)
        smat = np.zeros((nchunk, P, BLK), fp8)
        srcs[ch, sl] = r.astype(np.int16)
        smat[ch, sl, off_of[c]] = w.astype(fp8)
        shards[key] = (srcs, smat, (blk_of, off_of))
    return kp, sched, npairs, shards


def _pack_blocks(cfg, cnt, caps):
    """Assign QUART dests into NBLK blocks of exactly BLK dests so block
    edge-counts fit caps. Greedy: heaviest dest -> block with most slack."""
    nb = cfg.NBLK
    order = np.argsort(-cnt, kind='stable')
    load = np.zeros(nb, np.int64)
    nmem = np.zeros(nb, np.int64)
    blk_of = np.zeros(cfg.QUART, np.int64)
    off_of = np.zeros(cfg.QUART, np.int64)
    for d in order:
        slack = caps - load - cnt[d]
        slack[nmem >= BLK] = np.iinfo(np.int64).min
        j = int(np.argmax(slack))
        if slack[j] < 0:
            return None
        blk_of[d] = j
        off_of[d] = nmem[j]
        load[j] += cnt[d]
        nmem[j] += 1
    assert (nmem == BLK).all()
    return blk_of, off_of


def _wrap_idxs(srcs):
    """[C, 128] int16 -> wrapped [128, C*8] (idx j at [j%16 + 16*rep, j//16])."""
    flat = srcs.reshape(-1)
    w = flat.reshape(-1, 16).T                # [16, C*8]
    return np.ascontiguousarray(np.tile(w, (8, 1)).astype(np.int16))


def _build_scs(srcs, smat):
    """Combine wrapped idxs + grouped smat into one [calls, 128, 1152] u8."""
    nchunk = srcs.shape[0]
    calls = nchunk // GRP
    idxw = _wrap_idxs(srcs)                   # [128, nchunk*8] int16
    scs = np.zeros((calls, P, 128 + GRP * BLK), np.uint8)
    idxu = idxw.view(np.uint8).reshape(P, calls, 128).transpose(1, 0, 2)
    scs[:, :, 0:128] = idxu
    smu = smat.view(np.uint8).reshape(calls, GRP, P, BLK)
    scs[:, :, 128:] = smu.transpose(0, 2, 1, 3).reshape(calls, P, GRP * BLK)
    return scs


def _mpool(cfg, batch, qu, assign):
    """[128, NBLK, B] bf16 folding 1/cnt, zero rows for pad nodes."""
    batch = np.asarray(batch).astype(np.int64)
    cnt = np.bincount(batch, minlength=cfg.B).astype(np.float64)
    cinv = (1.0 / np.maximum(cnt, 1.0)).astype(np.float32)
    blk_of, off_of = assign
    m = np.zeros((P, cfg.NBLK, cfg.B), np.float32)
    lo = qu * cfg.QUART
    hi = min(lo + cfg.QUART, cfg.N)
    if hi > lo:
        nodes = np.arange(lo, hi)
        rel = nodes - lo
        m[off_of[rel], blk_of[rel], batch[nodes]] = cinv[batch[nodes]]
    return m.astype(_bf16())


def _xquant(cfg, x):
    """x [N, F] f32 -> padded [NPAD, F] fp8 (gather source rows)."""
    fp8 = _fp8()
    x8 = np.zeros((cfg.NPAD, cfg.F), fp8)
    x8[:cfg.N] = np.asarray(x, np.float32).astype(fp8)
    return x8


def _pack_arena(cfg, arrays):
    offs, total = _arena_offsets(cfg)
    ab = (total + 63) // 64 * 64
    arena = np.zeros((P, ab), np.uint8)
    for name, (off, rows, key, shape, nb) in offs.items():
        a = arrays[name]
        assert a.shape == (rows,) + tuple(shape), (name, a.shape, rows, shape)
        npdt = {'f32': np.float32, 'bf16': _bf16(), 'fp8': _fp8()}[key]
        flat = np.ascontiguousarray(a.astype(npdt)).view(np.uint8).reshape(rows, nb)
        arena[:rows, off:off + nb] = flat
    return arena


def _preprocess(inputs, cfg):
    fp8 = _fp8()
    kp, sched, npairs, shards = _edge_plan(
        cfg, {1: inputs['pro1_edge_index'], 2: inputs['pro2_edge_index']})
    meta = {'kp': tuple(int(v) for v in kp), 'sched': sched, 'npairs': npairs}

    def f32(v):
        return np.asarray(v, np.float32)

    xg = {br: _xquant(cfg, inputs[f'pro{br}_x']) for br in (1, 2)}
    in_maps = []
    for core in range(N_CORES):
        br, qu = core // N_QUART + 1, core % N_QUART
        ar = {}
        Wg = f32(inputs[f'W_g{br}'])
        ar['wg'] = np.ascontiguousarray(
            Wg.reshape(cfg.KC, P, cfg.F).transpose(1, 0, 2)).astype(fp8)
        ar['bg'] = np.tile(f32(inputs[f'b_g{br}'])[None, :], (P, 1))
        srcs, smat, assign = shards[(br, qu)]
        ar['mp'] = _mpool(cfg, inputs[f'pro{br}_batch'], qu, assign)
        Wpf = f32(inputs[f'W_pf{br}'])
        ar['wpf'] = np.ascontiguousarray(
            Wpf.reshape(cfg.KC, P, P).transpose(1, 0, 2))
        for mi in (1, 2):
            ar[f'wm{mi}'] = np.ascontiguousarray(
                (f32(inputs[f'W_m{mi}']) / (2.0 * cfg.LW))
                .reshape(8, 10, 64).transpose(1, 0, 2))
            ar[f'bm{mi}'] = f32(inputs[f'b_m{mi}']).reshape(64, 1)
            for sf, pre in (('s', 'cs'), ('f', 'cf')):
                w = float(np.asarray(inputs[f'{pre}{mi}_w'])[0])
                b = float(np.asarray(inputs[f'{pre}{mi}_b'])[0])
                ar[f'scale_{sf}{mi}'] = np.full((32, 1), w / cfg.C, np.float32)
                ar[f'bias_{sf}{mi}'] = np.full((32, 1), b, np.float32)
        gm = np.zeros((64, cfg.B), np.float32)
        gm[:, core * cfg.GPB:(core + 1) * cfg.GPB] = 1.0
        ar['gmask'] = gm

        arena = _pack_arena(cfg, ar)

        # masif arena2: 4 tensors x [32, 16, 100] f32 = 4 x 6400B per row
        a2 = np.zeros((32, 4 * 6400), np.uint8)
        for ti, name in enumerate(['mas1_straight', 'mas1_flipped',
                                   'mas2_straight', 'mas2_flipped']):
            a = f32(inputs[name])[core * cfg.GPB:(core + 1) * cfg.GPB]
            blk = a.reshape(cfg.GPB, cfg.C, cfg.LB, cfg.LBS) \
                   .transpose(2, 0, 1, 3).reshape(32, cfg.C * cfg.LBS)
            a2[:, ti * 6400:(ti + 1) * 6400] = \
                np.ascontiguousarray(blk).view(np.uint8)

        m = {'arena': arena, 'arena2': a2,
             'xg': xg[br],
             'scs': _build_scs(srcs, smat)}
        in_maps.append(m)
    return meta, in_maps


# ---------------------------------------------------------------- program
def _build(cfg, meta):
    import concourse.bass as bass
    import concourse.bacc as bacc
    import concourse.mybir as mybir
    import concourse.tile as tile
    from concourse.masks import make_identity

    dt = mybir.dt
    fp8 = dt.float8e4
    bf16 = dt.bfloat16
    f32 = dt.float32
    u8 = dt.uint8
    AF = mybir.ActivationFunctionType
    OP = mybir.AluOpType
    DR = mybir.MatmulPerfMode.DoubleRow

    nc = bacc.Bacc("TRN2", target_bir_lowering=False, debug=False,
                   enable_asserts=False, num_devices=N_CORES,
                   num_swdge_queues=2)

    offs, total = _arena_offsets(cfg)
    AB = (total + 63) // 64 * 64
    npairs = meta['npairs']
    sched = meta['sched']
    n_call = npairs // PAIRS_PER_CALL

    arena_d = nc.dram_tensor('arena', [P, AB], u8, kind="ExternalInput")
    arena2_d = nc.dram_tensor('arena2', [32, 4 * 6400], u8, kind="ExternalInput")
    xg_d = nc.dram_tensor('xg', [cfg.NPAD, cfg.F], fp8, kind="ExternalInput")
    scs_d = nc.dram_tensor('scs', [n_call, P, 128 + GRP * BLK], u8,
                           kind="ExternalInput")
    out_t = nc.dram_tensor('out', [P, 64], f32, kind="ExternalOutput")

    with tile.TileContext(nc) as tc:
        with tc.tile_pool(name="const", bufs=1) as cst, \
             tc.tile_pool(name="scs", bufs=12) as scsp, \
             tc.tile_pool(name="gat", bufs=8) as gatp, \
             tc.tile_pool(name="aggps", bufs=3, space="PSUM") as aggpsp, \
             tc.tile_pool(name="aggsb", bufs=2) as aggsbp, \
             tc.tile_pool(name="hb", bufs=3) as hp, \
             tc.tile_pool(name="poolps", bufs=1, space="PSUM") as poolp, \
             tc.tile_pool(name="small", bufs=2) as smp, \
             tc.tile_pool(name="smallps", bufs=1, space="PSUM") as smps:

            # ---------------- constant arena (one DMA each)
            arena_t = cst.tile([P, AB], u8, tag='arena')
            nc.sync.dma_start(out=arena_t[:], in_=arena_d.ap())
            arena2_t = cst.tile([32, 4 * 6400], u8, tag='arena2')
            nc.sync.dma_start(out=arena2_t[:], in_=arena2_d.ap())

            def av(name, dtype):
                off, rows, key, shape, nb = offs[name]
                v = arena_t[0:rows, off:off + nb].bitcast(dtype)
                if len(shape) == 2:
                    v = v.rearrange("p (a b) -> p a b", a=shape[0])
                return v

            wg_v = av('wg', fp8)
            bg_v = av('bg', f32)
            mp_v = av('mp', bf16)
            wpf_v = av('wpf', f32)
            wm_v = {mi: av(f'wm{mi}', f32) for mi in (1, 2)}
            bm_v = {mi: av(f'bm{mi}', f32) for mi in (1, 2)}
            msc_v = {(mi, sf, kind): av(f'{kind}_{sf}{mi}', f32)
                     for mi in (1, 2) for sf in 'sf'
                     for kind in ('scale', 'bias')}
            gmask_v = av('gmask', f32)

            id32 = cst.tile([32, 32], f32, tag='id32')
            make_identity(nc, id32[:])

            # ---------------- masif (both branches) -> two [64, B] f32 tiles
            for mi in (1, 2):
                frag = None
                for si, sf in enumerate('sf'):
                    toff = ((mi - 1) * 2 + si) * 6400
                    mv = arena2_t[:, toff:toff + 6400].bitcast(f32) \
                        .rearrange("p (c l) -> p c l", c=cfg.C)
                    red = smp.tile([32, cfg.LBS], f32, tag='masred')
                    nc.vector.tensor_reduce(
                        out=red[:], in_=mv.transpose([0, 2, 1]),
                        axis=mybir.AxisListType.X, op=OP.add)
                    act = smp.tile([32, cfg.LBS], f32, tag='masact')
                    nc.scalar.activation(
                        act[:], red[:], AF.Relu,
                        bias=msc_v[(mi, sf, 'bias')][:, 0:1],
                        scale=msc_v[(mi, sf, 'scale')][:, 0:1])
                    ws = smp.tile([32, cfg.WPB], f32, tag='masws')
                    nc.vector.tensor_reduce(
                        out=ws[:],
                        in_=act[:].rearrange("p (w l) -> p w l", l=cfg.LW),
                        axis=mybir.AxisListType.X, op=OP.add)
                    if frag is None:
                        frag = ws
                    else:
                        frag2 = smp.tile([32, cfg.WPB], f32, tag='masfrag')
                        nc.vector.tensor_add(out=frag2[:], in0=frag[:],
                                             in1=ws[:])
                        frag = frag2
                ps_t = smps.tile([cfg.WPB, 32], f32, space="PSUM", tag='sps')
                nc.tensor.transpose(out=ps_t[:], in_=frag[:], identity=id32[:])
                fragT = smp.tile([cfg.WPB, 32], f32, tag='masfragT')
                nc.scalar.activation(fragT[:], ps_t[:], AF.Identity)
                fragTc = fragT[:].rearrange("k (lb g) -> k lb g", g=cfg.GPB)
                m_ps = smps.tile([64, cfg.GPB], f32, space="PSUM", tag='sps')
                for lb in range(cfg.LB):
                    nc.tensor.matmul(
                        m_ps[:], lhsT=wm_v[mi][:, lb, :], rhs=fragTc[:, lb, :],
                        start=(lb == 0), stop=(lb == cfg.LB - 1))
                m_fm = smp.tile([64, cfg.GPB], f32, tag='masfm')
                nc.scalar.activation(m_fm[:], m_ps[:], AF.Identity,
                                     bias=bm_v[mi][:, 0:1])
                masm = cst.tile([64, cfg.B], f32, tag=f'masasm{mi}')
                nc.vector.tensor_tensor(
                    out=masm[:].rearrange("p (s g) -> p s g", g=cfg.GPB),
                    in0=m_fm[:, None, :].to_broadcast(
                        [64, N_CORES, cfg.GPB]),
                    in1=gmask_v.rearrange("p (s g) -> p s g", g=cfg.GPB),
                    op=OP.mult)
                # scalar queue: keeps the sync queue free for scs prefetch
                nc.scalar.dma_start(
                    out=out_t.ap()[(mi - 1) * 64:mi * 64, 32:64], in_=masm[:])

            # ---------------- GCN branch (this core's branch + quarter)
            # aggregate-first: aggT[f, d] = sum_e x[src_e, f] * S[e, d]
            # gathered straight from the x input tensor (no producer dep),
            # then h = lrelu(aggT^T @ W + b) per 128-dest block.
            poolps = poolp.tile([P, cfg.KC, cfg.B], f32, space="PSUM",
                                tag='poolps')
            pi = 0
            agg_ps = None
            for g in range(n_call):
                scs_t = scsp.tile([P, 128 + GRP * BLK], u8, tag='scs')
                nc.sync.dma_start(out=scs_t[:], in_=scs_d.ap()[g])
                idx_v = scs_t[:, 0:128].bitcast(dt.int16)
                s_v = scs_t[:, 128:128 + GRP * BLK].bitcast(fp8) \
                    .rearrange("p (c d) -> p c d", c=GRP)
                gat_t = gatp.tile([P, GRP, cfg.F], fp8, tag='gat')
                nc.gpsimd.dma_gather(
                    out_ap=gat_t[:], in_ap=xg_d.ap(),
                    idxs_ap=idx_v,
                    num_idxs=GRP * P, num_idxs_reg=GRP * P,
                    elem_size=cfg.F, queue_num=g % 2)
                for i in range(PAIRS_PER_CALL):
                    j, st, sp = sched[pi]
                    if st:
                        agg_ps = aggpsp.tile([P, cfg.KC, BLK], f32,
                                             space="PSUM", tag='aggps')
                    for c in range(cfg.KC):
                        nc.tensor.matmul(
                            agg_ps[:, c, :],
                            lhsT=gat_t[:, 2 * i:2 * i + 2,
                                       c * P:(c + 1) * P],
                            rhs=s_v[:, 2 * i:2 * i + 2, :],
                            start=st, stop=sp, perf_mode=DR,
                            skip_group_check=True)
                    if sp:
                        agg_sb = aggsbp.tile([P, cfg.KC, BLK], fp8,
                                             tag='aggsb')
                        if j % 2 == 0:
                            nc.vector.tensor_scalar_mul(
                                agg_sb[:], agg_ps[:], 1.0)
                        else:
                            nc.scalar.activation(
                                agg_sb[:], agg_ps[:], AF.Identity)
                        h_ps = aggpsp.tile([P, cfg.F], f32, space="PSUM",
                                           tag='aggps')
                        for c in range(cfg.KP):
                            for hf in range(2):
                                nc.tensor.matmul(
                                    h_ps[:, hf * 512:(hf + 1) * 512],
                                    lhsT=agg_sb[:, 2 * c:2 * c + 2, :],
                                    rhs=wg_v[:, 2 * c:2 * c + 2,
                                             hf * 512:(hf + 1) * 512],
                                    start=(c == 0), stop=(c == cfg.KP - 1),
                                    perf_mode=DR, skip_group_check=True)
                        h_t = hp.tile([P, cfg.F], bf16, tag='h')
                        nc.vector.tensor_add(out=h_t[:], in0=h_ps[:],
                                             in1=bg_v[:])
                        nc.scalar.activation(h_t[:], h_t[:], AF.Lrelu,
                                             alpha=0.01)
                        for c in range(cfg.KC):
                            nc.tensor.matmul(
                                poolps[:, c, :],
                                lhsT=h_t[:, c * P:(c + 1) * P],
                                rhs=mp_v[:, j, :],
                                start=(j == 0), stop=(j == cfg.NBLK - 1),
                                skip_group_check=True)
                    pi += 1

            # x_pre partial: [128, B] = W_pf^T @ pooledT
            pooled_sb = smp.tile([P, cfg.KC, cfg.B], f32, tag='pooled')
            nc.vector.tensor_scalar_mul(pooled_sb[:], poolps[:], 1.0)
            xpre_ps = smps.tile([P, cfg.B], f32, space="PSUM", tag='sps')
            for c in range(cfg.KC):
                nc.tensor.matmul(xpre_ps[:], lhsT=wpf_v[:, c, :],
                                 rhs=pooled_sb[:, c, :],
                                 start=(c == 0), stop=(c == cfg.KC - 1))
            xpre_sb = smp.tile([P, cfg.B], f32, tag='xpresb')
            nc.vector.tensor_scalar_mul(xpre_sb[:], xpre_ps[:], 1.0)
            nc.sync.dma_start(out=out_t.ap()[:, 0:32], in_=xpre_sb[:])

    nc.compile()
    return nc


# ---------------------------------------------------------------- entry
_CACHE = {}


def _run(inputs, cfg, trace=False, tmpdir=None):
    from concourse import bass_utils
    meta, in_maps = _preprocess(inputs, cfg)
    key = (cfg.N, cfg.F, meta['npairs'], meta['kp'])
    if key not in _CACHE:
        _CACHE.clear()
        _CACHE[key] = _build(cfg, meta)
    nc = _CACHE[key]
    res = bass_utils.run_bass_kernel_spmd(
        nc, in_maps, core_ids=list(range(N_CORES)), trace=trace, tmpdir=tmpdir)
    outs = [np.asarray(res.results[i]['out'], np.float64)
            for i in range(N_CORES)]
    out = _host_head(inputs, cfg, outs)
    return out, res


def _lrelu(v):
    return np.where(v > 0, v, 0.01 * v)


def _host_head(inputs, cfg, outs):
    """Unshard: sum of per-core partials -> tiny dense head (host)."""
    f32 = np.float32
    xr1 = sum(outs[i][:, 0:32] for i in range(4))
    xr2 = sum(outs[i][:, 0:32] for i in range(4, 8))
    m1 = sum(outs[i][0:64, 32:64] for i in range(N_CORES))
    m2 = sum(outs[i][64:128, 32:64] for i in range(N_CORES))
    x1 = _lrelu(xr1 + np.asarray(inputs['b_pf1'], f32)[:, None])
    x2 = _lrelu(xr2 + np.asarray(inputs['b_pf2'], f32)[:, None])
    xcat = np.concatenate([x1, x2], 0)                       # [256, B]
    xc1 = _lrelu(np.asarray(inputs['W_fc1'], f32).T @ xcat
                 + np.asarray(inputs['b_fc1'], f32)[:, None])
    xc2 = _lrelu(np.asarray(inputs['W_fc2'], f32).T @ xc1
                 + np.asarray(inputs['b_fc2'], f32)[:, None])
    W_out = np.asarray(inputs['W_out'], f32)
    z = (W_out[0:64].T @ xc2 + W_out[64:128].T @ m1 + W_out[128:192].T @ m2
         + np.asarray(inputs['b_out'], f32)[:, None])
    return (1.0 / (1.0 + np.exp(-z))).T.astype(f32)         # [B, 1]


def kernel(**inputs) -> np.ndarray:
    cfg = _Cfg()
    out, _ = _run(inputs, cfg)
    return out


# revision 27
# speedup vs baseline: 1.6487x; 1.0076x over previous
"""Trainium2 Bass kernel for nn_GCNN_87668872446200 (v3: branch-split + AllGather).

Two GCNConv+pool protein branches + two masif conv branches + dense head,
distributed over 8 NeuronCores as 2 branch-groups x 4 dest-node quarters.

Cores 0-3 handle protein branch 1 (quarters 0-3), cores 4-7 branch 2.
Per core (full 1024-dim features on the heavy paths, fp8):
  - xw = x_quarter @ W via fp8 DoubleRow matmuls for ONLY this core's 2560
    nodes -> local DRAM [2560, 1024] fp8 (no redundant compute)
  - ONE AllGather per 4-core branch group -> xw_full [10240, 1024] fp8
  - dma_gather pulls full 1KB source rows for this core's edge quarter
    (half the descriptors vs 512B rows, better HBM efficiency)
  - scatter-add as fp8 DoubleRow PE matmuls: S[256 edges, 128 dests]
    (host-built, norm-scaled, degree-balanced dest blocks) x gathered
    [256, 1024] accumulated in PSUM (2 x 512 halves)
  - h = lrelu(psum + bias) [128, 1024] bf16; transposed mean-pool via PE
    (pooledT [1024, 32]) folding 1/cnt
  - x_pre = W_pf^T @ pooledT partial [128, 32] (pre-activation, linear ->
    summable across the 4 quarter cores on host)
  - masif branch: 4 graphs/core (all cores, same as before)
  - out [128, 64]: cols 0:32 xpre partial, cols 32:64 masif partials
Host: sum partials per branch group, run the tiny dense head.

All 8 cores run ONE identical program; per-core variation is in input data.
"""
import numpy as np

# ---------------------------------------------------------------- constants
N_CORES = 8
N_QUART = 4       # dest-node quarters per branch group
P = 128
BLK = 128         # dest nodes per scatter block (S width)
PAIR_E = 256      # edges per DoubleRow matmul (2 chunks of 128)
GRP = 8           # chunks per dma_gather call (1024 idxs per call)
PAIRS_PER_CALL = GRP // 2

# problem sizes (hardcoded per spec)
N_NODES, N_EDGES, F_DIM, B_GRAPHS, L_MAS, C_MAS = 10000, 80000, 1024, 32, 800, 16


def _fp8():
    import ml_dtypes
    return ml_dtypes.float8_e4m3fn


def _bf16():
    import ml_dtypes
    return ml_dtypes.bfloat16


class _Cfg:
    def __init__(self, n=N_NODES, e=N_EDGES, f=F_DIM, b=B_GRAPHS,
                 l=L_MAS, c=C_MAS):
        assert f % 512 == 0 and b == 32 and l % 80 == 0 and c % 2 == 0
        self.N, self.E, self.F, self.B, self.L, self.C = n, e, f, b, l, c
        self.NPAD = ((n + 2047) // 2048) * 2048
        self.QUART = self.NPAD // N_QUART      # nodes per quarter (2560)
        assert self.QUART % 512 == 0
        self.NT = self.QUART // 512            # local node tiles (5)
        self.NBLK = self.QUART // BLK          # dest blocks per quarter (20)
        self.KC = f // P                       # k-chunks of contraction (8)
        self.KP = self.KC // 2                 # k-pairs (DoubleRow) (4)
        self.GPB = b // N_CORES                # graphs per core for masif
        self.LW = l // 80                      # avg-pool window (10)
        self.LB = 8                            # l-blocks for masif layout
        self.LBS = l // self.LB                # l-block size (100)
        assert self.LBS % self.LW == 0
        self.WPB = self.LBS // self.LW         # windows per l-block (10)


# ------------------------------------------------------------- arena layout
# (name, rows, dtype-key, shape) -- shared by host packer and kernel views
def _arena_layout(cfg):
    return [
        ('wg', 128, 'fp8', (cfg.KC, cfg.F)),
        ('bg', 128, 'f32', (cfg.F,)),
        ('mp', 128, 'bf16', (cfg.NBLK, cfg.B)),
        ('wpf', 128, 'f32', (cfg.KC, 128)),
        ('wm1', 10, 'f32', (8, 64)),
        ('wm2', 10, 'f32', (8, 64)),
        ('bm1', 64, 'f32', (1,)),
        ('bm2', 64, 'f32', (1,)),
        ('scale_s1', 32, 'f32', (1,)),
        ('bias_s1', 32, 'f32', (1,)),
        ('scale_f1', 32, 'f32', (1,)),
        ('bias_f1', 32, 'f32', (1,)),
        ('scale_s2', 32, 'f32', (1,)),
        ('bias_s2', 32, 'f32', (1,)),
        ('scale_f2', 32, 'f32', (1,)),
        ('bias_f2', 32, 'f32', (1,)),
        ('gmask', 64, 'f32', (cfg.B,)),
    ]


def _dt_size(key):
    return {'f32': 4, 'bf16': 2, 'fp8': 1}[key]


def _arena_offsets(cfg):
    off, out = 0, {}
    for name, rows, key, shape in _arena_layout(cfg):
        nb = int(np.prod(shape)) * _dt_size(key)
        out[name] = (off, rows, key, shape, nb)
        off += (nb + 63) // 64 * 64
    return out, off


# ---------------------------------------------------------------- host prep
def _edge_plan(cfg, edge_indices):
    """Shared block-pair profile (kp) + per-shard scatter plans.

    Returns (kp, sched, npairs, shards) where shards[(br, qu)] =
    (srcs [nchunk,128] int16, smat [nchunk,128,128] fp8, dest_of) and
    dest_of[(br, qu)] maps block,offset -> global dest node (for mp).
    """
    fp8 = _fp8()
    per_shard = {}
    for br in (1, 2):
        ei = edge_indices[br]
        row = np.asarray(ei[0]).astype(np.int64)
        col = np.asarray(ei[1]).astype(np.int64)
        loops = np.arange(cfg.N, dtype=np.int64)
        rows = np.concatenate([row, loops])
        cols = np.concatenate([col, loops])
        deg = np.bincount(cols, minlength=cfg.N).astype(np.float64)
        dinv = 1.0 / np.sqrt(deg)
        norm = (dinv[rows] * dinv[cols]).astype(np.float32)
        for qu in range(N_QUART):
            lo = qu * cfg.QUART
            sel = (cols >= lo) & (cols < lo + cfg.QUART)
            per_shard[(br, qu)] = (rows[sel], cols[sel] - lo, norm[sel])

    e_max = max(len(r) for r, _, _ in per_shard.values())
    npairs = -(-int(e_max * 1.02) // PAIR_E)
    npairs = -(-npairs // PAIRS_PER_CALL) * PAIRS_PER_CALL

    # find a feasible shared kp profile (block capacities) for all shards
    while True:
        base, extra = npairs // cfg.NBLK, npairs % cfg.NBLK
        kp = np.full(cfg.NBLK, base, np.int64)
        kp[:extra] += 1
        caps = kp * PAIR_E
        ok = True
        assigns = {}
        for key, (r, c, w) in per_shard.items():
            cnt = np.bincount(c, minlength=cfg.QUART)
            a = _pack_blocks(cfg, cnt, caps)
            if a is None:
                ok = False
                break
            assigns[key] = a
        if ok:
            break
        npairs += PAIRS_PER_CALL

    sched = []
    for j in range(cfg.NBLK):
        for t in range(kp[j]):
            sched.append((j, t == 0, t == int(kp[j]) - 1))
    assert len(sched) == npairs
    base_ch = np.zeros(cfg.NBLK, np.int64)
    base_ch[1:] = np.cumsum(2 * kp)[:-1]
    nchunk = 2 * npairs

    shards = {}
    for key, (r, c, w) in per_shard.items():
        blk_of, off_of = assigns[key]          # per local dest
        order = np.lexsort((off_of[c], blk_of[c]))
        r, c, w = r[order], c[order], w[order]
        b = blk_of[c]
        starts = np.searchsorted(b, np.arange(cfg.NBLK), 'left')
        o = np.arange(len(r)) - starts[b]
        ch = base_ch[b] + o // P
        sl = o % P
        srcs = np.zeros((nchunk, P), np.int16)
        smat = np.zeros((nchunk, P, BLK), fp8)
        srcs[ch, sl] = r.astype(np.int16)
        smat[ch, sl, off_of[c]] = w.astype(fp8)
        shards[key] = (srcs, smat, (blk_of, off_of))
    return kp, sched, npairs, shards


def _pack_blocks(cfg, cnt, caps):
    """Assign QUART dests into NBLK blocks of exactly BLK dests so block
    edge-counts fit caps. Greedy: heaviest dest -> block with most slack."""
    nb = cfg.NBLK
    order = np.argsort(-cnt, kind='stable')
    load = np.zeros(nb, np.int64)
    nmem = np.zeros(nb, np.int64)
    blk_of = np.zeros(cfg.QUART, np.int64)
    off_of = np.zeros(cfg.QUART, np.int64)
    for d in order:
        slack = caps - load - cnt[d]
        slack[nmem >= BLK] = np.iinfo(np.int64).min
        j = int(np.argmax(slack))
        if slack[j] < 0:
            return None
        blk_of[d] = j
        off_of[d] = nmem[j]
        load[j] += cnt[d]
        nmem[j] += 1
    assert (nmem == BLK).all()
    return blk_of, off_of


def _wrap_idxs(srcs):
    """[C, 128] int16 -> wrapped [128, C*8] (idx j at [j%16 + 16*rep, j//16])."""
    flat = srcs.reshape(-1)
    w = flat.reshape(-1, 16).T                # [16, C*8]
    return np.ascontiguousarray(np.tile(w, (8, 1)).astype(np.int16))


def _build_scs(srcs, smat):
    """Combine wrapped idxs + grouped smat into one [calls, 128, 1152] u8."""
    nchunk = srcs.shape[0]
    calls = nchunk // GRP
    idxw = _wrap_idxs(srcs)                   # [128, nchunk*8] int16
    scs = np.zeros((calls, P, 128 + GRP * BLK), np.uint8)
    idxu = idxw.view(np.uint8).reshape(P, calls, 128).transpose(1, 0, 2)
    scs[:, :, 0:128] = idxu
    smu = smat.view(np.uint8).reshape(calls, GRP, P, BLK)
    scs[:, :, 128:] = smu.transpose(0, 2, 1, 3).reshape(calls, P, GRP * BLK)
    return scs


def _mpool(cfg, batch, qu, assign):
    """[128, NBLK, B] bf16 folding 1/cnt, zero rows for pad nodes."""
    batch = np.asarray(batch).astype(np.int64)
    cnt = np.bincount(batch, minlength=cfg.B).astype(np.float64)
    cinv = (1.0 / np.maximum(cnt, 1.0)).astype(np.float32)
    blk_of, off_of = assign
    m = np.zeros((P, cfg.NBLK, cfg.B), np.float32)
    lo = qu * cfg.QUART
    hi = min(lo + cfg.QUART, cfg.N)
    if hi > lo:
        nodes = np.arange(lo, hi)
        rel = nodes - lo
        m[off_of[rel], blk_of[rel], batch[nodes]] = cinv[batch[nodes]]
    return m.astype(_bf16())


def _xquant(cfg, x):
    """x [N, F] f32 -> padded [NPAD, F] fp8 (gather source rows)."""
    fp8 = _fp8()
    x8 = np.zeros((cfg.NPAD, cfg.F), fp8)
    x8[:cfg.N] = np.asarray(x, np.float32).astype(fp8)
    return x8


def _pack_arena(cfg, arrays):
    offs, total = _arena_offsets(cfg)
    ab = (total + 63) // 64 * 64
    arena = np.zeros((P, ab), np.uint8)
    for name, (off, rows, key, shape, nb) in offs.items():
        a = arrays[name]
        assert a.shape == (rows,) + tuple(shape), (name, a.shape, rows, shape)
        npdt = {'f32': np.float32, 'bf16': _bf16(), 'fp8': _fp8()}[key]
        flat = np.ascontiguousarray(a.astype(npdt)).view(np.uint8).reshape(rows, nb)
        arena[:rows, off:off + nb] = flat
    return arena


def _preprocess(inputs, cfg):
    fp8 = _fp8()
    kp, sched, npairs, shards = _edge_plan(
        cfg, {1: inputs['pro1_edge_index'], 2: inputs['pro2_edge_index']})
    meta = {'kp': tuple(int(v) for v in kp), 'sched': sched, 'npairs': npairs}

    def f32(v):
        return np.asarray(v, np.float32)

    xg = {br: _xquant(cfg, inputs[f'pro{br}_x']) for br in (1, 2)}
    in_maps = []
    for core in range(N_CORES):
        br, qu = core // N_QUART + 1, core % N_QUART
        ar = {}
        Wg = f32(inputs[f'W_g{br}'])
        ar['wg'] = np.ascontiguousarray(
            Wg.reshape(cfg.KC, P, cfg.F).transpose(1, 0, 2)).astype(fp8)
        ar['bg'] = np.tile(f32(inputs[f'b_g{br}'])[None, :], (P, 1))
        srcs, smat, assign = shards[(br, qu)]
        ar['mp'] = _mpool(cfg, inputs[f'pro{br}_batch'], qu, assign)
        Wpf = f32(inputs[f'W_pf{br}'])
        ar['wpf'] = np.ascontiguousarray(
            Wpf.reshape(cfg.KC, P, P).transpose(1, 0, 2))
        for mi in (1, 2):
            ar[f'wm{mi}'] = np.ascontiguousarray(
                (f32(inputs[f'W_m{mi}']) / (2.0 * cfg.LW))
                .reshape(8, 10, 64).transpose(1, 0, 2))
            ar[f'bm{mi}'] = f32(inputs[f'b_m{mi}']).reshape(64, 1)
            for sf, pre in (('s', 'cs'), ('f', 'cf')):
                w = float(np.asarray(inputs[f'{pre}{mi}_w'])[0])
                b = float(np.asarray(inputs[f'{pre}{mi}_b'])[0])
                ar[f'scale_{sf}{mi}'] = np.full((32, 1), w / cfg.C, np.float32)
                ar[f'bias_{sf}{mi}'] = np.full((32, 1), b, np.float32)
        gm = np.zeros((64, cfg.B), np.float32)
        gm[:, core * cfg.GPB:(core + 1) * cfg.GPB] = 1.0
        ar['gmask'] = gm

        arena = _pack_arena(cfg, ar)

        # masif arena2: 4 tensors x [32, 16, 100] f32 = 4 x 6400B per row
        a2 = np.zeros((32, 4 * 6400), np.uint8)
        for ti, name in enumerate(['mas1_straight', 'mas1_flipped',
                                   'mas2_straight', 'mas2_flipped']):
            a = f32(inputs[name])[core * cfg.GPB:(core + 1) * cfg.GPB]
            blk = a.reshape(cfg.GPB, cfg.C, cfg.LB, cfg.LBS) \
                   .transpose(2, 0, 1, 3).reshape(32, cfg.C * cfg.LBS)
            a2[:, ti * 6400:(ti + 1) * 6400] = \
                np.ascontiguousarray(blk).view(np.uint8)

        m = {'arena': arena, 'arena2': a2,
             'xg': xg[br],
             'scs': _build_scs(srcs, smat)}
        in_maps.append(m)
    return meta, in_maps


# ---------------------------------------------------------------- program
def _build(cfg, meta):
    import concourse.bass as bass
    import concourse.bacc as bacc
    import concourse.mybir as mybir
    import concourse.tile as tile
    from concourse.masks import make_identity

    dt = mybir.dt
    fp8 = dt.float8e4
    bf16 = dt.bfloat16
    f32 = dt.float32
    u8 = dt.uint8
    AF = mybir.ActivationFunctionType
    OP = mybir.AluOpType
    DR = mybir.MatmulPerfMode.DoubleRow

    nc = bacc.Bacc("TRN2", target_bir_lowering=False, debug=False,
                   enable_asserts=False, num_devices=N_CORES,
                   num_swdge_queues=2)

    offs, total = _arena_offsets(cfg)
    AB = (total + 63) // 64 * 64
    npairs = meta['npairs']
    sched = meta['sched']
    n_call = npairs // PAIRS_PER_CALL

    arena_d = nc.dram_tensor('arena', [P, AB], u8, kind="ExternalInput")
    arena2_d = nc.dram_tensor('arena2', [32, 4 * 6400], u8, kind="ExternalInput")
    xg_d = nc.dram_tensor('xg', [cfg.NPAD, cfg.F], fp8, kind="ExternalInput")
    scs_d = nc.dram_tensor('scs', [n_call, P, 128 + GRP * BLK], u8,
                           kind="ExternalInput")
    out_t = nc.dram_tensor('out', [P, 64], f32, kind="ExternalOutput")

    with tile.TileContext(nc) as tc:
        with tc.tile_pool(name="const", bufs=1) as cst, \
             tc.tile_pool(name="scs", bufs=12) as scsp, \
             tc.tile_pool(name="gat", bufs=10) as gatp, \
             tc.tile_pool(name="aggps", bufs=3, space="PSUM") as aggpsp, \
             tc.tile_pool(name="aggsb", bufs=2) as aggsbp, \
             tc.tile_pool(name="hb", bufs=3) as hp, \
             tc.tile_pool(name="poolps", bufs=1, space="PSUM") as poolp, \
             tc.tile_pool(name="small", bufs=2) as smp, \
             tc.tile_pool(name="smallps", bufs=1, space="PSUM") as smps:

            # scs prefetch FIRST on the sync DMA queue: the first gather only
            # needs scs[0], not the 3.7MB arena transfers
            scs_tiles = {}
            for g in range(min(6, n_call)):
                t = scsp.tile([P, 128 + GRP * BLK], u8, tag='scs')
                nc.sync.dma_start(out=t[:], in_=scs_d.ap()[g])
                scs_tiles[g] = t

            # ---------------- constant arena (one DMA each)
            arena_t = cst.tile([P, AB], u8, tag='arena')
            nc.sync.dma_start(out=arena_t[:], in_=arena_d.ap())
            arena2_t = cst.tile([32, 4 * 6400], u8, tag='arena2')
            nc.sync.dma_start(out=arena2_t[:], in_=arena2_d.ap())

            def av(name, dtype):
                off, rows, key, shape, nb = offs[name]
                v = arena_t[0:rows, off:off + nb].bitcast(dtype)
                if len(shape) == 2:
                    v = v.rearrange("p (a b) -> p a b", a=shape[0])
                return v

            wg_v = av('wg', fp8)
            bg_v = av('bg', f32)
            mp_v = av('mp', bf16)
            wpf_v = av('wpf', f32)
            wm_v = {mi: av(f'wm{mi}', f32) for mi in (1, 2)}
            bm_v = {mi: av(f'bm{mi}', f32) for mi in (1, 2)}
            msc_v = {(mi, sf, kind): av(f'{kind}_{sf}{mi}', f32)
                     for mi in (1, 2) for sf in 'sf'
                     for kind in ('scale', 'bias')}
            gmask_v = av('gmask', f32)

            id32 = cst.tile([32, 32], f32, tag='id32')
            make_identity(nc, id32[:])

            # ---------------- masif (both branches) -> two [64, B] f32 tiles
            for mi in (1, 2):
                frag = None
                for si, sf in enumerate('sf'):
                    toff = ((mi - 1) * 2 + si) * 6400
                    mv = arena2_t[:, toff:toff + 6400].bitcast(f32) \
                        .rearrange("p (c l) -> p c l", c=cfg.C)
                    red = smp.tile([32, cfg.LBS], f32, tag='masred')
                    nc.vector.tensor_reduce(
                        out=red[:], in_=mv.transpose([0, 2, 1]),
                        axis=mybir.AxisListType.X, op=OP.add)
                    act = smp.tile([32, cfg.LBS], f32, tag='masact')
                    nc.scalar.activation(
                        act[:], red[:], AF.Relu,
                        bias=msc_v[(mi, sf, 'bias')][:, 0:1],
                        scale=msc_v[(mi, sf, 'scale')][:, 0:1])
                    ws = smp.tile([32, cfg.WPB], f32, tag='masws')
                    nc.vector.tensor_reduce(
                        out=ws[:],
                        in_=act[:].rearrange("p (w l) -> p w l", l=cfg.LW),
                        axis=mybir.AxisListType.X, op=OP.add)
                    if frag is None:
                        frag = ws
                    else:
                        frag2 = smp.tile([32, cfg.WPB], f32, tag='masfrag')
                        nc.vector.tensor_add(out=frag2[:], in0=frag[:],
                                             in1=ws[:])
                        frag = frag2
                ps_t = smps.tile([cfg.WPB, 32], f32, space="PSUM", tag='sps')
                nc.tensor.transpose(out=ps_t[:], in_=frag[:], identity=id32[:])
                fragT = smp.tile([cfg.WPB, 32], f32, tag='masfragT')
                nc.scalar.activation(fragT[:], ps_t[:], AF.Identity)
                fragTc = fragT[:].rearrange("k (lb g) -> k lb g", g=cfg.GPB)
                m_ps = smps.tile([64, cfg.GPB], f32, space="PSUM", tag='sps')
                for lb in range(cfg.LB):
                    nc.tensor.matmul(
                        m_ps[:], lhsT=wm_v[mi][:, lb, :], rhs=fragTc[:, lb, :],
                        start=(lb == 0), stop=(lb == cfg.LB - 1))
                m_fm = smp.tile([64, cfg.GPB], f32, tag='masfm')
                nc.scalar.activation(m_fm[:], m_ps[:], AF.Identity,
                                     bias=bm_v[mi][:, 0:1])
                masm = cst.tile([64, cfg.B], f32, tag=f'masasm{mi}')
                nc.vector.tensor_tensor(
                    out=masm[:].rearrange("p (s g) -> p s g", g=cfg.GPB),
                    in0=m_fm[:, None, :].to_broadcast(
                        [64, N_CORES, cfg.GPB]),
                    in1=gmask_v.rearrange("p (s g) -> p s g", g=cfg.GPB),
                    op=OP.mult)
                # scalar queue: keeps the sync queue free for scs prefetch
                nc.scalar.dma_start(
                    out=out_t.ap()[(mi - 1) * 64:mi * 64, 32:64], in_=masm[:])

            # ---------------- GCN branch (this core's branch + quarter)
            # aggregate-first: aggT[f, d] = sum_e x[src_e, f] * S[e, d]
            # gathered straight from the x input tensor (no producer dep),
            # then h = lrelu(aggT^T @ W + b) per 128-dest block.
            poolps = poolp.tile([P, cfg.KC, cfg.B], f32, space="PSUM",
                                tag='poolps')

            def emit_pool(j, h_t):
                for c in range(cfg.KC):
                    nc.tensor.matmul(
                        poolps[:, c, :],
                        lhsT=h_t[:, c * P:(c + 1) * P],
                        rhs=mp_v[:, j, :],
                        start=(j == 0), stop=(j == cfg.NBLK - 1),
                        skip_group_check=True)

            pi = 0
            agg_ps = None
            pending_pool = None
            for g in range(n_call):
                if g in scs_tiles:
                    scs_t = scs_tiles[g]
                else:
                    scs_t = scsp.tile([P, 128 + GRP * BLK], u8, tag='scs')
                    nc.sync.dma_start(out=scs_t[:], in_=scs_d.ap()[g])
                idx_v = scs_t[:, 0:128].bitcast(dt.int16)
                s_v = scs_t[:, 128:128 + GRP * BLK].bitcast(fp8) \
                    .rearrange("p (c d) -> p c d", c=GRP)
                gat_t = gatp.tile([P, GRP, cfg.F], fp8, tag='gat')
                nc.gpsimd.dma_gather(
                    out_ap=gat_t[:], in_ap=xg_d.ap(),
                    idxs_ap=idx_v,
                    num_idxs=GRP * P, num_idxs_reg=GRP * P,
                    elem_size=cfg.F, queue_num=g % 2)
                for i in range(PAIRS_PER_CALL):
                    j, st, sp = sched[pi]
                    if st:
                        agg_ps = aggpsp.tile([P, cfg.KC, BLK], f32,
                                             space="PSUM", tag='aggps')
                    for c in range(cfg.KC):
                        nc.tensor.matmul(
                            agg_ps[:, c, :],
                            lhsT=gat_t[:, 2 * i:2 * i + 2,
                                       c * P:(c + 1) * P],
                            rhs=s_v[:, 2 * i:2 * i + 2, :],
                            start=st, stop=sp, perf_mode=DR,
                            skip_group_check=True)
                    if sp:
                        # pool matmuls for the PREVIOUS block land here, after
                        # ~5 pairs of agg matmuls hid the h chain latency
                        if pending_pool is not None:
                            emit_pool(*pending_pool)
                        agg_sb = aggsbp.tile([P, cfg.KC, BLK], fp8,
                                             tag='aggsb')
                        if j % 2 == 0:
                            nc.vector.tensor_scalar_mul(
                                agg_sb[:], agg_ps[:], 1.0)
                        else:
                            nc.scalar.activation(
                                agg_sb[:], agg_ps[:], AF.Identity)
                        h_ps = aggpsp.tile([P, cfg.F], f32, space="PSUM",
                                           tag='aggps')
                        for c in range(cfg.KP):
                            for hf in range(2):
                                nc.tensor.matmul(
                                    h_ps[:, hf * 512:(hf + 1) * 512],
                                    lhsT=agg_sb[:, 2 * c:2 * c + 2, :],
                                    rhs=wg_v[:, 2 * c:2 * c + 2,
                                             hf * 512:(hf + 1) * 512],
                                    start=(c == 0), stop=(c == cfg.KP - 1),
                                    perf_mode=DR, skip_group_check=True)
                        h_t = hp.tile([P, cfg.F], bf16, tag='h')
                        nc.vector.tensor_add(out=h_t[:], in0=h_ps[:],
                                             in1=bg_v[:])
                        nc.scalar.activation(h_t[:], h_t[:], AF.Lrelu,
                                             alpha=0.01)
                        pending_pool = (j, h_t)
                    pi += 1
            emit_pool(*pending_pool)

            # x_pre partial: [128, B] = W_pf^T @ pooledT
            pooled_sb = smp.tile([P, cfg.KC, cfg.B], f32, tag='pooled')
            nc.vector.tensor_scalar_mul(pooled_sb[:], poolps[:], 1.0)
            xpre_ps = smps.tile([P, cfg.B], f32, space="PSUM", tag='sps')
            for c in range(cfg.KC):
                nc.tensor.matmul(xpre_ps[:], lhsT=wpf_v[:, c, :],
                                 rhs=pooled_sb[:, c, :],
                                 start=(c == 0), stop=(c == cfg.KC - 1))
            xpre_sb = smp.tile([P, cfg.B], f32, tag='xpresb')
            nc.vector.tensor_scalar_mul(xpre_sb[:], xpre_ps[:], 1.0)
            nc.sync.dma_start(out=out_t.ap()[:, 0:32], in_=xpre_sb[:])

    nc.compile()
    return nc


# ---------------------------------------------------------------- entry
_CACHE = {}


def _run(inputs, cfg, trace=False, tmpdir=None):
    from concourse import bass_utils
    meta, in_maps = _preprocess(inputs, cfg)
    key = (cfg.N, cfg.F, meta['npairs'], meta['kp'])
    if key not in _CACHE:
        _CACHE.clear()
        _CACHE[key] = _build(cfg, meta)
    nc = _CACHE[key]
    res = bass_utils.run_bass_kernel_spmd(
        nc, in_maps, core_ids=list(range(N_CORES)), trace=trace, tmpdir=tmpdir)
    outs = [np.asarray(res.results[i]['out'], np.float64)
            for i in range(N_CORES)]
    out = _host_head(inputs, cfg, outs)
    return out, res


def _lrelu(v):
    return np.where(v > 0, v, 0.01 * v)


def _host_head(inputs, cfg, outs):
    """Unshard: sum of per-core partials -> tiny dense head (host)."""
    f32 = np.float32
    xr1 = sum(outs[i][:, 0:32] for i in range(4))
    xr2 = sum(outs[i][:, 0:32] for i in range(4, 8))
    m1 = sum(outs[i][0:64, 32:64] for i in range(N_CORES))
    m2 = sum(outs[i][64:128, 32:64] for i in range(N_CORES))
    x1 = _lrelu(xr1 + np.asarray(inputs['b_pf1'], f32)[:, None])
    x2 = _lrelu(xr2 + np.asarray(inputs['b_pf2'], f32)[:, None])
    xcat = np.concatenate([x1, x2], 0)                       # [256, B]
    xc1 = _lrelu(np.asarray(inputs['W_fc1'], f32).T @ xcat
                 + np.asarray(inputs['b_fc1'], f32)[:, None])
    xc2 = _lrelu(np.asarray(inputs['W_fc2'], f32).T @ xc1
                 + np.asarray(inputs['b_fc2'], f32)[:, None])
    W_out = np.asarray(inputs['W_out'], f32)
    z = (W_out[0:64].T @ xc2 + W_out[64:128].T @ m1 + W_out[128:192].T @ m2
         + np.asarray(inputs['b_out'], f32)[:, None])
    return (1.0 / (1.0 + np.exp(-z))).T.astype(f32)         # [B, 1]


def kernel(**inputs) -> np.ndarray:
    cfg = _Cfg()
    out, _ = _run(inputs, cfg)
    return out


# revision 30
# speedup vs baseline: 1.7407x; 1.0558x over previous
"""Trainium2 Bass kernel for nn_GCNN_87668872446200 (v3: branch-split + AllGather).

Two GCNConv+pool protein branches + two masif conv branches + dense head,
distributed over 8 NeuronCores as 2 branch-groups x 4 dest-node quarters.

Cores 0-3 handle protein branch 1 (quarters 0-3), cores 4-7 branch 2.
Per core (full 1024-dim features on the heavy paths, fp8):
  - xw = x_quarter @ W via fp8 DoubleRow matmuls for ONLY this core's 2560
    nodes -> local DRAM [2560, 1024] fp8 (no redundant compute)
  - ONE AllGather per 4-core branch group -> xw_full [10240, 1024] fp8
  - dma_gather pulls full 1KB source rows for this core's edge quarter
    (half the descriptors vs 512B rows, better HBM efficiency)
  - scatter-add as fp8 DoubleRow PE matmuls: S[256 edges, 128 dests]
    (host-built, norm-scaled, degree-balanced dest blocks) x gathered
    [256, 1024] accumulated in PSUM (2 x 512 halves)
  - h = lrelu(psum + bias) [128, 1024] bf16; transposed mean-pool via PE
    (pooledT [1024, 32]) folding 1/cnt
  - x_pre = W_pf^T @ pooledT partial [128, 32] (pre-activation, linear ->
    summable across the 4 quarter cores on host)
  - masif branch: 4 graphs/core (all cores, same as before)
  - out [128, 64]: cols 0:32 xpre partial, cols 32:64 masif partials
Host: sum partials per branch group, run the tiny dense head.

All 8 cores run ONE identical program; per-core variation is in input data.
"""
import numpy as np

# ---------------------------------------------------------------- constants
N_CORES = 8
N_QUART = 4       # dest-node quarters per branch group
P = 128
BLK = 128         # dest nodes per scatter block (S width)
PAIR_E = 256      # edges per DoubleRow matmul (2 chunks of 128)
GRP = 8           # chunks per dma_gather call (1024 idxs per call)
PAIRS_PER_CALL = GRP // 2

# problem sizes (hardcoded per spec)
N_NODES, N_EDGES, F_DIM, B_GRAPHS, L_MAS, C_MAS = 10000, 80000, 1024, 32, 800, 16


def _fp8():
    import ml_dtypes
    return ml_dtypes.float8_e4m3fn


def _bf16():
    import ml_dtypes
    return ml_dtypes.bfloat16


class _Cfg:
    def __init__(self, n=N_NODES, e=N_EDGES, f=F_DIM, b=B_GRAPHS,
                 l=L_MAS, c=C_MAS):
        assert f % 512 == 0 and b == 32 and l % 80 == 0 and c % 2 == 0
        self.N, self.E, self.F, self.B, self.L, self.C = n, e, f, b, l, c
        self.NPAD = ((n + 2047) // 2048) * 2048
        self.QUART = self.NPAD // N_QUART      # nodes per quarter (2560)
        assert self.QUART % 512 == 0
        self.NT = self.QUART // 512            # local node tiles (5)
        self.NBLK = self.QUART // BLK          # dest blocks per quarter (20)
        self.KC = f // P                       # k-chunks of contraction (8)
        self.KP = self.KC // 2                 # k-pairs (DoubleRow) (4)
        self.GPB = b // N_CORES                # graphs per core for masif
        self.LW = l // 80                      # avg-pool window (10)
        self.LB = 8                            # l-blocks for masif layout
        self.LBS = l // self.LB                # l-block size (100)
        assert self.LBS % self.LW == 0
        self.WPB = self.LBS // self.LW         # windows per l-block (10)


# ------------------------------------------------------------- arena layout
# (name, rows, dtype-key, shape) -- shared by host packer and kernel views
def _arena_layout(cfg):
    return [
        ('wg', 128, 'fp8', (cfg.KC, cfg.F)),
        ('bg', 128, 'f32', (cfg.F,)),
        ('mp', 128, 'bf16', (cfg.NBLK, cfg.B)),
        ('wpf', 128, 'f32', (cfg.KC, 128)),
        ('wm1', 10, 'f32', (8, 64)),
        ('wm2', 10, 'f32', (8, 64)),
        ('bm1', 64, 'f32', (1,)),
        ('bm2', 64, 'f32', (1,)),
        ('scale_s1', 32, 'f32', (1,)),
        ('bias_s1', 32, 'f32', (1,)),
        ('scale_f1', 32, 'f32', (1,)),
        ('bias_f1', 32, 'f32', (1,)),
        ('scale_s2', 32, 'f32', (1,)),
        ('bias_s2', 32, 'f32', (1,)),
        ('scale_f2', 32, 'f32', (1,)),
        ('bias_f2', 32, 'f32', (1,)),
        ('gmask', 64, 'f32', (cfg.B,)),
    ]


def _dt_size(key):
    return {'f32': 4, 'bf16': 2, 'fp8': 1}[key]


def _arena_offsets(cfg):
    off, out = 0, {}
    for name, rows, key, shape in _arena_layout(cfg):
        nb = int(np.prod(shape)) * _dt_size(key)
        out[name] = (off, rows, key, shape, nb)
        off += (nb + 63) // 64 * 64
    return out, off


# ---------------------------------------------------------------- host prep
def _edge_plan(cfg, edge_indices):
    """Shared block-pair profile (kp) + per-shard scatter plans.

    Returns (kp, sched, npairs, shards) where shards[(br, qu)] =
    (srcs [nchunk,128] int16, smat [nchunk,128,128] fp8, dest_of) and
    dest_of[(br, qu)] maps block,offset -> global dest node (for mp).
    """
    fp8 = _fp8()
    per_shard = {}
    for br in (1, 2):
        ei = edge_indices[br]
        row = np.asarray(ei[0]).astype(np.int64)
        col = np.asarray(ei[1]).astype(np.int64)
        loops = np.arange(cfg.N, dtype=np.int64)
        rows = np.concatenate([row, loops])
        cols = np.concatenate([col, loops])
        deg = np.bincount(cols, minlength=cfg.N).astype(np.float64)
        dinv = 1.0 / np.sqrt(deg)
        norm = (dinv[rows] * dinv[cols]).astype(np.float32)
        for qu in range(N_QUART):
            lo = qu * cfg.QUART
            sel = (cols >= lo) & (cols < lo + cfg.QUART)
            per_shard[(br, qu)] = (rows[sel], cols[sel] - lo, norm[sel])

    e_max = max(len(r) for r, _, _ in per_shard.values())
    npairs = -(-int(e_max * 1.02) // PAIR_E)
    npairs = -(-npairs // PAIRS_PER_CALL) * PAIRS_PER_CALL

    # find a feasible shared kp profile (block capacities) for all shards
    while True:
        base, extra = npairs // cfg.NBLK, npairs % cfg.NBLK
        kp = np.full(cfg.NBLK, base, np.int64)
        kp[:extra] += 1
        caps = kp * PAIR_E
        ok = True
        assigns = {}
        for key, (r, c, w) in per_shard.items():
            cnt = np.bincount(c, minlength=cfg.QUART)
            a = _pack_blocks(cfg, cnt, caps)
            if a is None:
                ok = False
                break
            assigns[key] = a
        if ok:
            break
        npairs += PAIRS_PER_CALL

    sched = []
    for j in range(cfg.NBLK):
        for t in range(kp[j]):
            sched.append((j, t == 0, t == int(kp[j]) - 1))
    assert len(sched) == npairs
    base_ch = np.zeros(cfg.NBLK, np.int64)
    base_ch[1:] = np.cumsum(2 * kp)[:-1]
    nchunk = 2 * npairs

    shards = {}
    for key, (r, c, w) in per_shard.items():
        blk_of, off_of = assigns[key]          # per local dest
        order = np.lexsort((off_of[c], blk_of[c]))
        r, c, w = r[order], c[order], w[order]
        b = blk_of[c]
        starts = np.searchsorted(b, np.arange(cfg.NBLK), 'left')
        o = np.arange(len(r)) - starts[b]
        ch = base_ch[b] + o // P
        sl = o % P
        srcs = np.zeros((nchunk, P), np.int16)
        smat = np.zeros((nchunk, P, BLK), fp8)
        srcs[ch, sl] = r.astype(np.int16)
        smat[ch, sl, off_of[c]] = w.astype(fp8)
        shards[key] = (srcs, smat, (blk_of, off_of))
    return kp, sched, npairs, shards


def _pack_blocks(cfg, cnt, caps):
    """Assign QUART dests into NBLK blocks of exactly BLK dests so block
    edge-counts fit caps. Greedy: heaviest dest -> block with most slack."""
    nb = cfg.NBLK
    order = np.argsort(-cnt, kind='stable')
    load = np.zeros(nb, np.int64)
    nmem = np.zeros(nb, np.int64)
    blk_of = np.zeros(cfg.QUART, np.int64)
    off_of = np.zeros(cfg.QUART, np.int64)
    for d in order:
        slack = caps - load - cnt[d]
        slack[nmem >= BLK] = np.iinfo(np.int64).min
        j = int(np.argmax(slack))
        if slack[j] < 0:
            return None
        blk_of[d] = j
        off_of[d] = nmem[j]
        load[j] += cnt[d]
        nmem[j] += 1
    assert (nmem == BLK).all()
    return blk_of, off_of


def _wrap_idxs(srcs):
    """[C, 128] int16 -> wrapped [128, C*8] (idx j at [j%16 + 16*rep, j//16])."""
    flat = srcs.reshape(-1)
    w = flat.reshape(-1, 16).T                # [16, C*8]
    return np.ascontiguousarray(np.tile(w, (8, 1)).astype(np.int16))


def _build_scs(srcs, smat):
    """Combine wrapped idxs + grouped smat into one [calls, 128, 1152] u8."""
    nchunk = srcs.shape[0]
    calls = nchunk // GRP
    idxw = _wrap_idxs(srcs)                   # [128, nchunk*8] int16
    scs = np.zeros((calls, P, 128 + GRP * BLK), np.uint8)
    idxu = idxw.view(np.uint8).reshape(P, calls, 128).transpose(1, 0, 2)
    scs[:, :, 0:128] = idxu
    smu = smat.view(np.uint8).reshape(calls, GRP, P, BLK)
    scs[:, :, 128:] = smu.transpose(0, 2, 1, 3).reshape(calls, P, GRP * BLK)
    return scs


def _mpool(cfg, batch, qu, assign):
    """[128, NBLK, B] bf16 folding 1/cnt, zero rows for pad nodes."""
    batch = np.asarray(batch).astype(np.int64)
    cnt = np.bincount(batch, minlength=cfg.B).astype(np.float64)
    cinv = (1.0 / np.maximum(cnt, 1.0)).astype(np.float32)
    blk_of, off_of = assign
    m = np.zeros((P, cfg.NBLK, cfg.B), np.float32)
    lo = qu * cfg.QUART
    hi = min(lo + cfg.QUART, cfg.N)
    if hi > lo:
        nodes = np.arange(lo, hi)
        rel = nodes - lo
        m[off_of[rel], blk_of[rel], batch[nodes]] = cinv[batch[nodes]]
    return m.astype(_bf16())


def _xquant(cfg, x):
    """x [N, F] f32 -> padded [NPAD, F] fp8 (gather source rows)."""
    fp8 = _fp8()
    x8 = np.zeros((cfg.NPAD, cfg.F), fp8)
    x8[:cfg.N] = np.asarray(x, np.float32).astype(fp8)
    return x8


def _pack_arena(cfg, arrays):
    offs, total = _arena_offsets(cfg)
    ab = (total + 63) // 64 * 64
    arena = np.zeros((P, ab), np.uint8)
    for name, (off, rows, key, shape, nb) in offs.items():
        a = arrays[name]
        assert a.shape == (rows,) + tuple(shape), (name, a.shape, rows, shape)
        npdt = {'f32': np.float32, 'bf16': _bf16(), 'fp8': _fp8()}[key]
        flat = np.ascontiguousarray(a.astype(npdt)).view(np.uint8).reshape(rows, nb)
        arena[:rows, off:off + nb] = flat
    return arena


def _preprocess(inputs, cfg):
    fp8 = _fp8()
    kp, sched, npairs, shards = _edge_plan(
        cfg, {1: inputs['pro1_edge_index'], 2: inputs['pro2_edge_index']})
    meta = {'kp': tuple(int(v) for v in kp), 'sched': sched, 'npairs': npairs}

    def f32(v):
        return np.asarray(v, np.float32)

    xg = {br: _xquant(cfg, inputs[f'pro{br}_x']) for br in (1, 2)}
    in_maps = []
    for core in range(N_CORES):
        br, qu = core // N_QUART + 1, core % N_QUART
        ar = {}
        Wg = f32(inputs[f'W_g{br}'])
        ar['wg'] = np.ascontiguousarray(
            Wg.reshape(cfg.KC, P, cfg.F).transpose(1, 0, 2)).astype(fp8)
        ar['bg'] = np.tile(f32(inputs[f'b_g{br}'])[None, :], (P, 1))
        srcs, smat, assign = shards[(br, qu)]
        ar['mp'] = _mpool(cfg, inputs[f'pro{br}_batch'], qu, assign)
        Wpf = f32(inputs[f'W_pf{br}'])
        ar['wpf'] = np.ascontiguousarray(
            Wpf.reshape(cfg.KC, P, P).transpose(1, 0, 2))
        for mi in (1, 2):
            ar[f'wm{mi}'] = np.ascontiguousarray(
                (f32(inputs[f'W_m{mi}']) / (2.0 * cfg.LW))
                .reshape(8, 10, 64).transpose(1, 0, 2))
            ar[f'bm{mi}'] = f32(inputs[f'b_m{mi}']).reshape(64, 1)
            for sf, pre in (('s', 'cs'), ('f', 'cf')):
                w = float(np.asarray(inputs[f'{pre}{mi}_w'])[0])
                b = float(np.asarray(inputs[f'{pre}{mi}_b'])[0])
                ar[f'scale_{sf}{mi}'] = np.full((32, 1), w / cfg.C, np.float32)
                ar[f'bias_{sf}{mi}'] = np.full((32, 1), b, np.float32)
        gm = np.zeros((64, cfg.B), np.float32)
        gm[:, core * cfg.GPB:(core + 1) * cfg.GPB] = 1.0
        ar['gmask'] = gm

        arena = _pack_arena(cfg, ar)

        # masif arena2: 4 tensors x [32, 16, 100] f32 = 4 x 6400B per row
        a2 = np.zeros((32, 4 * 6400), np.uint8)
        for ti, name in enumerate(['mas1_straight', 'mas1_flipped',
                                   'mas2_straight', 'mas2_flipped']):
            a = f32(inputs[name])[core * cfg.GPB:(core + 1) * cfg.GPB]
            blk = a.reshape(cfg.GPB, cfg.C, cfg.LB, cfg.LBS) \
                   .transpose(2, 0, 1, 3).reshape(32, cfg.C * cfg.LBS)
            a2[:, ti * 6400:(ti + 1) * 6400] = \
                np.ascontiguousarray(blk).view(np.uint8)

        m = {'arena': arena, 'arena2': a2,
             'xg': xg[br],
             'scs': _build_scs(srcs, smat)}
        in_maps.append(m)
    return meta, in_maps


# ---------------------------------------------------------------- program
def _build(cfg, meta):
    import concourse.bass as bass
    import concourse.bacc as bacc
    import concourse.mybir as mybir
    import concourse.tile as tile
    from concourse.masks import make_identity

    dt = mybir.dt
    fp8 = dt.float8e4
    bf16 = dt.bfloat16
    f32 = dt.float32
    u8 = dt.uint8
    AF = mybir.ActivationFunctionType
    OP = mybir.AluOpType
    DR = mybir.MatmulPerfMode.DoubleRow

    nc = bacc.Bacc("TRN2", target_bir_lowering=False, debug=False,
                   enable_asserts=False, num_devices=N_CORES,
                   num_swdge_queues=2)

    offs, total = _arena_offsets(cfg)
    AB = (total + 63) // 64 * 64
    npairs = meta['npairs']
    sched = meta['sched']
    n_call = npairs // PAIRS_PER_CALL

    arena_d = nc.dram_tensor('arena', [P, AB], u8, kind="ExternalInput")
    arena2_d = nc.dram_tensor('arena2', [32, 4 * 6400], u8, kind="ExternalInput")
    xg_d = nc.dram_tensor('xg', [cfg.NPAD, cfg.F], fp8, kind="ExternalInput")
    scs_d = nc.dram_tensor('scs', [n_call, P, 128 + GRP * BLK], u8,
                           kind="ExternalInput")
    out_t = nc.dram_tensor('out', [P, 64], f32, kind="ExternalOutput")

    with tile.TileContext(nc) as tc:
        with tc.tile_pool(name="const", bufs=1) as cst, \
             tc.tile_pool(name="scs", bufs=12) as scsp, \
             tc.tile_pool(name="gat", bufs=10) as gatp, \
             tc.tile_pool(name="aggps", bufs=3, space="PSUM") as aggpsp, \
             tc.tile_pool(name="aggsb", bufs=3) as aggsbp, \
             tc.tile_pool(name="hb", bufs=3) as hp, \
             tc.tile_pool(name="poolps", bufs=1, space="PSUM") as poolp, \
             tc.tile_pool(name="small", bufs=2) as smp, \
             tc.tile_pool(name="smallps", bufs=1, space="PSUM") as smps:

            # scs prefetch FIRST on the sync DMA queue: the first gather only
            # needs scs[0], not the 3.7MB arena transfers
            scs_tiles = {}
            for g in range(min(6, n_call)):
                t = scsp.tile([P, 128 + GRP * BLK], u8, tag='scs')
                nc.sync.dma_start(out=t[:], in_=scs_d.ap()[g])
                scs_tiles[g] = t

            # ---------------- constant arena (one DMA each)
            arena_t = cst.tile([P, AB], u8, tag='arena')
            nc.sync.dma_start(out=arena_t[:], in_=arena_d.ap())
            arena2_t = cst.tile([32, 4 * 6400], u8, tag='arena2')
            nc.sync.dma_start(out=arena2_t[:], in_=arena2_d.ap())

            def av(name, dtype):
                off, rows, key, shape, nb = offs[name]
                v = arena_t[0:rows, off:off + nb].bitcast(dtype)
                if len(shape) == 2:
                    v = v.rearrange("p (a b) -> p a b", a=shape[0])
                return v

            wg_v = av('wg', fp8)
            bg_v = av('bg', f32)
            mp_v = av('mp', bf16)
            wpf_v = av('wpf', f32)
            wm_v = {mi: av(f'wm{mi}', f32) for mi in (1, 2)}
            bm_v = {mi: av(f'bm{mi}', f32) for mi in (1, 2)}
            msc_v = {(mi, sf, kind): av(f'{kind}_{sf}{mi}', f32)
                     for mi in (1, 2) for sf in 'sf'
                     for kind in ('scale', 'bias')}
            gmask_v = av('gmask', f32)

            id32 = cst.tile([32, 32], f32, tag='id32')
            make_identity(nc, id32[:])

            # ---------------- masif (both branches) -> two [64, B] f32 tiles
            for mi in (1, 2):
                frag = None
                for si, sf in enumerate('sf'):
                    toff = ((mi - 1) * 2 + si) * 6400
                    mv = arena2_t[:, toff:toff + 6400].bitcast(f32) \
                        .rearrange("p (c l) -> p c l", c=cfg.C)
                    red = smp.tile([32, cfg.LBS], f32, tag='masred')
                    nc.vector.tensor_reduce(
                        out=red[:], in_=mv.transpose([0, 2, 1]),
                        axis=mybir.AxisListType.X, op=OP.add)
                    act = smp.tile([32, cfg.LBS], f32, tag='masact')
                    nc.scalar.activation(
                        act[:], red[:], AF.Relu,
                        bias=msc_v[(mi, sf, 'bias')][:, 0:1],
                        scale=msc_v[(mi, sf, 'scale')][:, 0:1])
                    ws = smp.tile([32, cfg.WPB], f32, tag='masws')
                    nc.vector.tensor_reduce(
                        out=ws[:],
                        in_=act[:].rearrange("p (w l) -> p w l", l=cfg.LW),
                        axis=mybir.AxisListType.X, op=OP.add)
                    if frag is None:
                        frag = ws
                    else:
                        frag2 = smp.tile([32, cfg.WPB], f32, tag='masfrag')
                        nc.vector.tensor_add(out=frag2[:], in0=frag[:],
                                             in1=ws[:])
                        frag = frag2
                ps_t = smps.tile([cfg.WPB, 32], f32, space="PSUM", tag='sps')
                nc.tensor.transpose(out=ps_t[:], in_=frag[:], identity=id32[:])
                fragT = smp.tile([cfg.WPB, 32], f32, tag='masfragT')
                nc.scalar.activation(fragT[:], ps_t[:], AF.Identity)
                fragTc = fragT[:].rearrange("k (lb g) -> k lb g", g=cfg.GPB)
                m_ps = smps.tile([64, cfg.GPB], f32, space="PSUM", tag='sps')
                for lb in range(cfg.LB):
                    nc.tensor.matmul(
                        m_ps[:], lhsT=wm_v[mi][:, lb, :], rhs=fragTc[:, lb, :],
                        start=(lb == 0), stop=(lb == cfg.LB - 1))
                m_fm = smp.tile([64, cfg.GPB], f32, tag='masfm')
                nc.scalar.activation(m_fm[:], m_ps[:], AF.Identity,
                                     bias=bm_v[mi][:, 0:1])
                masm = cst.tile([64, cfg.B], f32, tag=f'masasm{mi}')
                nc.vector.tensor_tensor(
                    out=masm[:].rearrange("p (s g) -> p s g", g=cfg.GPB),
                    in0=m_fm[:, None, :].to_broadcast(
                        [64, N_CORES, cfg.GPB]),
                    in1=gmask_v.rearrange("p (s g) -> p s g", g=cfg.GPB),
                    op=OP.mult)
                # scalar queue: keeps the sync queue free for scs prefetch
                nc.scalar.dma_start(
                    out=out_t.ap()[(mi - 1) * 64:mi * 64, 32:64], in_=masm[:])

            # ---------------- GCN branch (this core's branch + quarter)
            # aggregate-first: aggT[f, d] = sum_e x[src_e, f] * S[e, d]
            # gathered straight from the x input tensor (no producer dep),
            # then h = lrelu(aggT^T @ W + b) per 128-dest block.
            poolps = poolp.tile([P, cfg.KC, cfg.B], f32, space="PSUM",
                                tag='poolps')

            def emit_pool(j, h_t):
                for c in range(cfg.KC):
                    nc.tensor.matmul(
                        poolps[:, c, :],
                        lhsT=h_t[:, c * P:(c + 1) * P],
                        rhs=mp_v[:, j, :],
                        start=(j == 0), stop=(j == cfg.NBLK - 1),
                        skip_group_check=True)

            def emit_h(j, agg_sb):
                h_ps = aggpsp.tile([P, cfg.F], f32, space="PSUM",
                                   tag='aggps')
                for c in range(cfg.KP):
                    for hf in range(2):
                        nc.tensor.matmul(
                            h_ps[:, hf * 512:(hf + 1) * 512],
                            lhsT=agg_sb[:, 2 * c:2 * c + 2, :],
                            rhs=wg_v[:, 2 * c:2 * c + 2,
                                     hf * 512:(hf + 1) * 512],
                            start=(c == 0), stop=(c == cfg.KP - 1),
                            perf_mode=DR, skip_group_check=True)
                h_t = hp.tile([P, cfg.F], bf16, tag='h')
                nc.vector.tensor_add(out=h_t[:], in0=h_ps[:], in1=bg_v[:])
                nc.scalar.activation(h_t[:], h_t[:], AF.Lrelu, alpha=0.01)
                return h_t

            # 2-deep software pipeline: at sp(j) emit copy(j), pool(j-2),
            # h(j-1) -- every consumed input is a full block old, so the PE
            # never stalls on the vector/scalar h chain.
            pi = 0
            agg_ps = None
            pending_h = None
            pending_pool = None
            for g in range(n_call):
                if g in scs_tiles:
                    scs_t = scs_tiles[g]
                else:
                    scs_t = scsp.tile([P, 128 + GRP * BLK], u8, tag='scs')
                    nc.sync.dma_start(out=scs_t[:], in_=scs_d.ap()[g])
                idx_v = scs_t[:, 0:128].bitcast(dt.int16)
                s_v = scs_t[:, 128:128 + GRP * BLK].bitcast(fp8) \
                    .rearrange("p (c d) -> p c d", c=GRP)
                gat_t = gatp.tile([P, GRP, cfg.F], fp8, tag='gat')
                nc.gpsimd.dma_gather(
                    out_ap=gat_t[:], in_ap=xg_d.ap(),
                    idxs_ap=idx_v,
                    num_idxs=GRP * P, num_idxs_reg=GRP * P,
                    elem_size=cfg.F, queue_num=g % 2)
                for i in range(PAIRS_PER_CALL):
                    j, st, sp = sched[pi]
                    if st:
                        agg_ps = aggpsp.tile([P, cfg.KC, BLK], f32,
                                             space="PSUM", tag='aggps')
                    for c in range(cfg.KC):
                        nc.tensor.matmul(
                            agg_ps[:, c, :],
                            lhsT=gat_t[:, 2 * i:2 * i + 2,
                                       c * P:(c + 1) * P],
                            rhs=s_v[:, 2 * i:2 * i + 2, :],
                            start=st, stop=sp, perf_mode=DR,
                            skip_group_check=True)
                    if sp:
                        agg_sb = aggsbp.tile([P, cfg.KC, BLK], fp8,
                                             tag='aggsb')
                        if j % 2 == 0:
                            nc.vector.tensor_scalar_mul(
                                agg_sb[:], agg_ps[:], 1.0)
                        else:
                            nc.scalar.activation(
                                agg_sb[:], agg_ps[:], AF.Identity)
                        if pending_pool is not None:
                            emit_pool(*pending_pool)
                            pending_pool = None
                        if pending_h is not None:
                            jh, sbh = pending_h
                            pending_pool = (jh, emit_h(jh, sbh))
                        pending_h = (j, agg_sb)
                    pi += 1
            if pending_h is not None:
                jh, sbh = pending_h
                if pending_pool is not None:
                    emit_pool(*pending_pool)
                    pending_pool = None
                last_pool = (jh, emit_h(jh, sbh))
                emit_pool(*last_pool)

            # x_pre partial: [128, B] = W_pf^T @ pooledT
            pooled_sb = smp.tile([P, cfg.KC, cfg.B], f32, tag='pooled')
            nc.vector.tensor_scalar_mul(pooled_sb[:], poolps[:], 1.0)
            xpre_ps = smps.tile([P, cfg.B], f32, space="PSUM", tag='sps')
            for c in range(cfg.KC):
                nc.tensor.matmul(xpre_ps[:], lhsT=wpf_v[:, c, :],
                                 rhs=pooled_sb[:, c, :],
                                 start=(c == 0), stop=(c == cfg.KC - 1))
            xpre_sb = smp.tile([P, cfg.B], f32, tag='xpresb')
            nc.vector.tensor_scalar_mul(xpre_sb[:], xpre_ps[:], 1.0)
            nc.sync.dma_start(out=out_t.ap()[:, 0:32], in_=xpre_sb[:])

    nc.compile()
    return nc


# ---------------------------------------------------------------- entry
_CACHE = {}


def _run(inputs, cfg, trace=False, tmpdir=None):
    from concourse import bass_utils
    meta, in_maps = _preprocess(inputs, cfg)
    key = (cfg.N, cfg.F, meta['npairs'], meta['kp'])
    if key not in _CACHE:
        _CACHE.clear()
        _CACHE[key] = _build(cfg, meta)
    nc = _CACHE[key]
    res = bass_utils.run_bass_kernel_spmd(
        nc, in_maps, core_ids=list(range(N_CORES)), trace=trace, tmpdir=tmpdir)
    outs = [np.asarray(res.results[i]['out'], np.float64)
            for i in range(N_CORES)]
    out = _host_head(inputs, cfg, outs)
    return out, res


def _lrelu(v):
    return np.where(v > 0, v, 0.01 * v)


def _host_head(inputs, cfg, outs):
    """Unshard: sum of per-core partials -> tiny dense head (host)."""
    f32 = np.float32
    xr1 = sum(outs[i][:, 0:32] for i in range(4))
    xr2 = sum(outs[i][:, 0:32] for i in range(4, 8))
    m1 = sum(outs[i][0:64, 32:64] for i in range(N_CORES))
    m2 = sum(outs[i][64:128, 32:64] for i in range(N_CORES))
    x1 = _lrelu(xr1 + np.asarray(inputs['b_pf1'], f32)[:, None])
    x2 = _lrelu(xr2 + np.asarray(inputs['b_pf2'], f32)[:, None])
    xcat = np.concatenate([x1, x2], 0)                       # [256, B]
    xc1 = _lrelu(np.asarray(inputs['W_fc1'], f32).T @ xcat
                 + np.asarray(inputs['b_fc1'], f32)[:, None])
    xc2 = _lrelu(np.asarray(inputs['W_fc2'], f32).T @ xc1
                 + np.asarray(inputs['b_fc2'], f32)[:, None])
    W_out = np.asarray(inputs['W_out'], f32)
    z = (W_out[0:64].T @ xc2 + W_out[64:128].T @ m1 + W_out[128:192].T @ m2
         + np.asarray(inputs['b_out'], f32)[:, None])
    return (1.0 / (1.0 + np.exp(-z))).T.astype(f32)         # [B, 1]


def kernel(**inputs) -> np.ndarray:
    cfg = _Cfg()
    out, _ = _run(inputs, cfg)
    return out
